# revision 1
# baseline (speedup 1.0000x reference)
"""3-layer GAT on Trainium2, 8 NeuronCores.

Strategy (dst-sharded, replicated tables):
- Nodes are remapped into 8 slices of 6656 rows (6250 real + pad); each core
  owns one slice of destination nodes and all edges pointing into it.
- Per layer, every core builds its slice of a node table
  [h(256) | Pl(8) | pl(8) | Pr(8) | pr(8) | pad] in bf16 (768B rows) where
  Pl=exp(el), pl=exp(0.2*el), Pr=exp(er), pr=exp(0.2*er); an AllGather
  replicates the full 53248-row table to every core.
- Edge phase per 128-dst window: dma_gather fetches the src rows for the
  window's edges (int16 indices; low/high table halves as two gathers),
  edge weights use exp(leaky_relu(el+er)) = max(Pl*Pr, pl*pr) (exact for
  slope<1), destination-side Pr/pr are expanded edge-wise with a one-hot
  matmul, and a second one-hot matmul segment-sums ex*h and ex into PSUM
  per dst. Softmax normalization happens after aggregation (out/s): exact
  because alpha = ex/sum(ex) is invariant to per-dst scaling (this also
  makes the explicit segment-max unnecessary; |logits| < 10 here).
- Layer 3 commutes the output projection with aggregation:
  sum(ex*h2) @ Wo == sum(ex*(h2@Wo)), with el3 = h2 @ (Wo@alo^T).
"""
import numpy as np
import ml_dtypes
from contextlib import ExitStack

import concourse.bass as bass
import concourse.tile as tile
from concourse import bacc, mybir
from concourse.bass_utils import run_bass_kernel_spmd
from concourse.masks import make_identity

BF16 = ml_dtypes.bfloat16

N_NODES = 50000
IN_F = 128
N_CLASSES = 40
CORES = 8
SPR = 6250          # real dst nodes per core
SP = 6656           # slice rows per core (52 * 128)
NT = SP * CORES     # 53248 padded table rows
HALF = 32768        # int16-addressable table half
W = 49              # dst windows per core (ceil(6250/128))
ROW = 384           # table row elems (bf16 -> 768B, multiple of 256B)
NTILE = SP // 128   # 52 node tiles per slice
EXPF = mybir.ActivationFunctionType.Exp


QT = NTILE // 4      # 13 tiles per collective quarter
QROWS = QT * 128     # 1664 slice rows per quarter
QBLK = QROWS * CORES # 13312 table rows per quarter


HT = NTILE // 2      # 26 tiles per collective half
HROWS = HT * 128     # 3328 slice rows per half
HBLK = HROWS * CORES # 26624 table rows per half


def _remap(n):
    """Global table row for node n, laid out (half, core, tile, row) so that
    half-wise partial AllGathers are contiguous in both the slice and the
    full table (AllGather concatenates per-core inputs)."""
    c = n // SPR
    r = n % SPR
    t = r // 128
    h = t // HT
    return h * HBLK + c * HROWS + (t % HT) * 128 + (r % 128)


def _wrap16(vals, nidx):
    """dma_gather index layout: flat idx i -> [i%16, i//16], replicated to
    all 8 groups of 16 partitions."""
    blk = np.zeros((16, nidx // 16), np.int16)
    blk[np.arange(nidx) % 16, np.arange(nidx) // 16] = vals
    return np.tile(blk, (8, 1))


def _host_prep(src, dst):
    gsrc = _remap(src.astype(np.int64))
    d64 = dst.astype(np.int64)
    core = d64 // SPR
    ld = d64 % SPR
    w = ld >> 7
    dstl = (ld & 127).astype(np.int64)
    ishigh = (gsrc >= HALF).astype(np.int64)
    wg = core * W + w

    order = np.lexsort((ishigh, wg))
    gsrc_s, dstl_s, wg_s, hi_s = gsrc[order], dstl[order], wg[order], ishigh[order]

    gkey = wg_s * 2 + hi_s
    uniq, starts, counts = np.unique(gkey, return_index=True, return_counts=True)
    pos_in_grp = np.arange(len(gkey)) - np.repeat(starts, counts)

    nlow = np.zeros(CORES * W, np.int64)
    nhigh = np.zeros(CORES * W, np.int64)
    for u, c in zip(uniq, counts):
        (nlow if u % 2 == 0 else nhigh)[u // 2] = c
    CL = int(np.ceil(nlow.max() / 128))
    CH = int(np.ceil(nhigh.max() / 128))
    CPW = CL + CH

    w_local = wg_s % W
    slot = w_local * (CPW * 128) + np.where(hi_s == 1, CL * 128, 0) + pos_in_grp
    core_s = wg_s // W

    NSLOT = W * CPW * 128
    per_core = []
    for c in range(CORES):
        m = core_s == c
        sl = slot[m]
        gs = gsrc_s[m]
        dl = dstl_s[m]
        hi = hi_s[m]

        lowidx = np.zeros(W * CL * 128, np.int16)
        highidx = np.zeros(W * CH * 128, np.int16)
        wl = sl // (CPW * 128)
        ks = sl % (CPW * 128)
        lm = hi == 0
        lowidx[wl[lm] * (CL * 128) + ks[lm]] = gs[lm].astype(np.int16)
        hm = hi == 1
        highidx[wl[hm] * (CH * 128) + (ks[hm] - CL * 128)] = (gs[hm] - HALF).astype(np.int16)

        gilo = np.concatenate(
            [_wrap16(lowidx[i * CL * 128:(i + 1) * CL * 128], CL * 128) for i in range(W)], axis=1)
        gihi = np.concatenate(
            [_wrap16(highidx[i * CH * 128:(i + 1) * CH * 128], CH * 128) for i in range(W)], axis=1)

        # one-hot streams; pad slots keep all-zero column/row -> zero contribution
        selS = np.zeros((128, NSLOT), BF16)    # sel[d, slot]
        selT = np.zeros((128, NSLOT), BF16)    # selT[e_lane, chunk*128 + d]
        e_lane = sl % 128
        chunk = sl // 128
        selS[dl, sl] = 1
        selT[e_lane, chunk * 128 + dl] = 1

        per_core.append(dict(gilo=gilo, gihi=gihi, selS=selS, selT=selT))
    return per_core, CL, CH, CPW


def _alar_block(al, ar, fout):
    """[fout, 16]: col j (<8) extracts el head j, col j+8 er head j."""
    H, F = al.shape
    m = np.zeros((fout, 16), np.float32)
    for j in range(H):
        m[j * F:(j + 1) * F, j] = al[j]
        m[j * F:(j + 1) * F, j + 8] = ar[j]
    return m


def _build_program(CL, CH, CPW):
    nc = bacc.Bacc("TRN2", target_bir_lowering=False, debug=False, num_devices=CORES)
    f32, bf16, i16 = mybir.dt.float32, mybir.dt.bfloat16, mybir.dt.int16

    xsl = nc.declare_dram_parameter("xsl", [SP, IN_F], f32, isOutput=False)
    gilo_d = nc.declare_dram_parameter("gilo", [128, W * CL * 8], i16, isOutput=False)
    gihi_d = nc.declare_dram_parameter("gihi", [128, W * CH * 8], i16, isOutput=False)
    selS_d = nc.declare_dram_parameter("selS", [128, W * CPW * 128], bf16, isOutput=False)
    selT_d = nc.declare_dram_parameter("selT", [128, W * CPW * 128], bf16, isOutput=False)
    mneg_d = nc.declare_dram_parameter("mneg", [1, SP], bf16, isOutput=False)
    w1_d = nc.declare_dram_parameter("w1", [IN_F, 256], bf16, isOutput=False)
    w2_d = nc.declare_dram_parameter("w2", [128, 2, 256], bf16, isOutput=False)
    wo_d = nc.declare_dram_parameter("wo", [128, 2, N_CLASSES], bf16, isOutput=False)
    alar_d = nc.declare_dram_parameter("alar", [128, 3, 2, 16], bf16, isOutput=False)
    outy = nc.declare_dram_parameter("outy", [SP, N_CLASSES], f32, isOutput=True)

    with ExitStack() as ctx:
        tc = ctx.enter_context(tile.TileContext(nc))
        const = ctx.enter_context(tc.tile_pool(name="const", bufs=1))
        dram = ctx.enter_context(tc.tile_pool(name="dram", bufs=1, space="DRAM"))
        gpool = ctx.enter_context(tc.tile_pool(name="gpool", bufs=2))
        spool = ctx.enter_context(tc.tile_pool(name="spool", bufs=2))
        npool = ctx.enter_context(tc.tile_pool(name="npool", bufs=2))
        pwin = ctx.enter_context(tc.tile_pool(name="pwin", bufs=2, space="PSUM"))
        pnode = ctx.enter_context(tc.tile_pool(name="pnode", bufs=2, space="PSUM"))

        Tsl_h = [dram.tile([HROWS, ROW], bf16, name="tsl0"),
                 dram.tile([HROWS, ROW], bf16, name="tsl1")]
        TfullA = dram.tile([NT, ROW], bf16)
        TfullB = dram.tile([NT, ROW], bf16)

        def Tslice_rows(r0, r1):
            h = r0 // HROWS
            assert (r1 - 1) // HROWS == h
            return Tsl_h[h][r0 - h * HROWS:r1 - h * HROWS, :]

        gilo_t = const.tile([128, W * CL * 8], i16)
        nc.sync.dma_start(out=gilo_t[:], in_=gilo_d[:, :])
        gihi_t = const.tile([128, W * CH * 8], i16)
        nc.sync.dma_start(out=gihi_t[:], in_=gihi_d[:, :])
        mneg_t = const.tile([1, SP], bf16)
        nc.sync.dma_start(out=mneg_t[:], in_=mneg_d[:, :])
        w1_t = const.tile([IN_F, 256], bf16)
        nc.sync.dma_start(out=w1_t[:], in_=w1_d[:, :])
        w2_t = const.tile([128, 2, 256], bf16)
        nc.sync.dma_start(out=w2_t[:], in_=w2_d[:, :, :])
        wo_t = const.tile([128, 2, N_CLASSES], bf16)
        nc.sync.dma_start(out=wo_t[:], in_=wo_d[:, :, :])
        alar_t = const.tile([128, 3, 2, 16], bf16)
        nc.sync.dma_start(out=alar_t[:], in_=alar_d[:, :, :, :])
        ident = const.tile([128, 128], bf16)
        make_identity(nc, ident[:])
        ones16 = const.tile([1, 16], bf16)
        nc.vector.memset(ones16[:], 1.0)

        def emit_table_rows(al_idx, h_T, h_node_src, w):
            """Assemble table row tile [h|Pl|pl|Pr|pr|0] for node rows
            [w*128,(w+1)*128) and DMA it into Tslice.
            al_idx: which alar block; h_T: [128,2,128] bf16 feature-major;
            h_node_src: AP with node-major h values [128, 256] (any float)."""
            row_t = npool.tile([128, ROW], bf16, tag="row")
            nc.vector.tensor_copy(out=row_t[:, 0:256], in_=h_node_src)
            el_ps = pnode.tile([16, 128], f32, tag="nps")
            for kt in range(2):
                nc.tensor.matmul(out=el_ps[:], lhsT=alar_t[:, al_idx, kt, :],
                                 rhs=h_T[:, kt, :], start=(kt == 0), stop=False)
            nc.tensor.matmul(out=el_ps[:], lhsT=ones16[:],
                             rhs=mneg_t[:, w * 128:(w + 1) * 128], start=False, stop=True)
            P_t = npool.tile([128, 128], bf16, tag="Pt")
            p_t = npool.tile([128, 128], bf16, tag="pt")
            nc.scalar.activation(out=P_t[0:16, :], in_=el_ps[:], func=EXPF, scale=1.0)
            nc.scalar.activation(out=p_t[0:16, :], in_=el_ps[:], func=EXPF, scale=0.2)
            Pt_ps = pnode.tile([128, 128], bf16, tag="nps")
            nc.tensor.transpose(out=Pt_ps[:], in_=P_t[:], identity=ident[:])
            nc.vector.tensor_copy(out=row_t[:, 256:264], in_=Pt_ps[:, 0:8])     # Pl
            nc.vector.tensor_copy(out=row_t[:, 272:280], in_=Pt_ps[:, 8:16])    # Pr
            pt_ps = pnode.tile([128, 128], bf16, tag="nps")
            nc.tensor.transpose(out=pt_ps[:], in_=p_t[:], identity=ident[:])
            nc.vector.tensor_copy(out=row_t[:, 264:272], in_=pt_ps[:, 0:8])     # pl
            nc.vector.tensor_copy(out=row_t[:, 280:288], in_=pt_ps[:, 8:16])    # pr
            nc.vector.memset(row_t[:, 288:ROW], 0.0)
            nc.sync.dma_start(out=Tslice_rows(w * 128, (w + 1) * 128), in_=row_t[:])

        # ---- P0: layer-1 table from x ----
        for t in range(NTILE):
            x_t = npool.tile([128, IN_F], f32, tag="xt")
            nc.sync.dma_start(out=x_t[:], in_=xsl[t * 128:(t + 1) * 128, :])
            xb = npool.tile([128, IN_F], bf16, tag="xb")
            nc.vector.tensor_copy(out=xb[:], in_=x_t[:])
            xT_ps = pnode.tile([128, 128], bf16, tag="nps")
            nc.tensor.transpose(out=xT_ps[:], in_=xb[:], identity=ident[:])
            xT = npool.tile([128, 128], bf16, tag="xT")
            nc.vector.tensor_copy(out=xT[:], in_=xT_ps[:])
            h_ps = pnode.tile([128, 2, 128], f32, tag="nps")
            for mt in range(2):
                nc.tensor.matmul(out=h_ps[:, mt, :], lhsT=w1_t[:, mt * 128:(mt + 1) * 128],
                                 rhs=xT[:], start=True, stop=True)
            h_T = npool.tile([128, 2, 128], bf16, tag="hT")
            nc.vector.tensor_copy(out=h_T[:], in_=h_ps[:])
            hb_ps = pnode.tile([128, 2, 128], bf16, tag="nps")
            for t2 in range(2):
                nc.tensor.transpose(out=hb_ps[:, t2, :], in_=h_T[:, t2, :], identity=ident[:])
            hb = npool.tile([128, 256], f32, tag="hb")
            nc.vector.tensor_copy(out=hb[:], in_=hb_ps[:].rearrange("p a b -> p (a b)"))
            emit_table_rows(0, h_T, hb[:], t)
            if t in (2 * QT - 1, NTILE - 1):
                h = 0 if t == 2 * QT - 1 else 1
                nc.gpsimd.collective_compute(
                    "AllGather", mybir.AluOpType.bypass,
                    replica_groups=[list(range(CORES))],
                    ins=[Tsl_h[h].opt()],
                    outs=[TfullA[h * HBLK:(h + 1) * HBLK, :]])

        # ---- 3 layers of windowed edge aggregation ----
        for l in range(3):
            Tf = TfullA if l % 2 == 0 else TfullB
            Tnext = TfullB if l % 2 == 0 else TfullA
            for w in range(W):
                g_win = gpool.tile([128, CPW, ROW], bf16, tag="gwin")
                nc.gpsimd.dma_gather(
                    out_ap=g_win[:, 0:CL, :], in_ap=Tf[0:HALF, :],
                    idxs_ap=gilo_t[:, w * CL * 8:(w + 1) * CL * 8],
                    num_idxs=CL * 128, num_idxs_reg=CL * 128, elem_size=ROW,
                    single_packet=False)
                nc.gpsimd.dma_gather(
                    out_ap=g_win[:, CL:CPW, :], in_ap=Tf[HALF:NT, :],
                    idxs_ap=gihi_t[:, w * CH * 8:(w + 1) * CH * 8],
                    num_idxs=CH * 128, num_idxs_reg=CH * 128, elem_size=ROW,
                    single_packet=False)
                selw = spool.tile([128, CPW, 128], bf16, tag="selw")
                nc.sync.dma_start(out=selw[:], in_=selS_d[:, w * CPW * 128:(w + 1) * CPW * 128])
                selTw = spool.tile([128, CPW, 128], bf16, tag="selTw")
                nc.sync.dma_start(out=selTw[:], in_=selT_d[:, w * CPW * 128:(w + 1) * CPW * 128])
                prpr = spool.tile([128, 16], bf16, tag="prpr")
                nc.sync.dma_start(out=prpr[:], in_=Tslice_rows(w * 128, (w + 1) * 128)[:, 272:288])

                # expand dst-side Pr/pr to edge-major: pp[e, 16]
                pp_ps = pwin.tile([128, CPW, 16], f32, tag="ppps")
                for c in range(CPW):
                    nc.tensor.matmul(out=pp_ps[:, c, :], lhsT=selw[:, c, :],
                                     rhs=prpr[:], start=True, stop=True)
                # ex = max(Pl*Pr, pl*pr)
                cand = spool.tile([128, CPW, 2, 8], bf16, tag="cand")
                nc.vector.tensor_tensor(
                    out=cand[:],
                    in0=g_win[:, :, 256:272].rearrange("p c (a h) -> p c a h", a=2),
                    in1=pp_ps[:].rearrange("p c (a h) -> p c a h", a=2),
                    op=mybir.AluOpType.mult)
                rhs_w = spool.tile([128, CPW, 264], bf16, tag="rhsw")
                nc.vector.tensor_tensor(
                    out=rhs_w[:, :, 256:264], in0=cand[:, :, 0, :], in1=cand[:, :, 1, :],
                    op=mybir.AluOpType.max)
                if l < 2:
                    nc.vector.tensor_tensor(
                        out=rhs_w[:, :, 0:256].rearrange("p c (h f) -> p c h f", h=8),
                        in0=g_win[:, :, 0:256].rearrange("p c (h f) -> p c h f", h=8),
                        in1=rhs_w[:, :, 256:264].rearrange("p c (h o) -> p c h o", o=1)
                            .to_broadcast([128, CPW, 8, 32]),
                        op=mybir.AluOpType.mult)
                else:
                    nc.vector.tensor_tensor(
                        out=rhs_w[:, :, 0:256],
                        in0=g_win[:, :, 0:256],
                        in1=rhs_w[:, :, 256:257].to_broadcast([128, CPW, 256]),
                        op=mybir.AluOpType.mult)
                agg_ps = pwin.tile([128, 264], f32, tag="aggps")
                for c in range(CPW):
                    nc.tensor.matmul(out=agg_ps[:], lhsT=selTw[:, c, :], rhs=rhs_w[:, c, :],
                                     start=(c == 0), stop=(c == CPW - 1))

                # ---- per-window node phase ----
                if l < 2:
                    s_rec = npool.tile([128, 8], f32, tag="srec")
                    nc.vector.tensor_scalar_add(s_rec[:], agg_ps[:, 256:264], 1e-16)
                    nc.vector.reciprocal(out=s_rec[:], in_=s_rec[:])
                    u_t = npool.tile([128, 256], f32, tag="ut")
                    nc.vector.tensor_tensor(
                        out=u_t[:].rearrange("p (h f) -> p h f", h=8),
                        in0=agg_ps[:, 0:256].rearrange("p (h f) -> p h f", h=8),
                        in1=s_rec[:].rearrange("p (h o) -> p h o", o=1).to_broadcast([128, 8, 32]),
                        op=mybir.AluOpType.mult)
                    # elu(x) = exp(min(x,0)) - 1 + relu(x)
                    m0 = npool.tile([128, 256], f32, tag="m0")
                    nc.vector.tensor_scalar_min(m0[:], u_t[:], 0.0)
                    e0 = npool.tile([128, 256], f32, tag="e0")
                    nc.scalar.activation(out=e0[:], in_=m0[:], func=EXPF)
                    r0 = npool.tile([128, 256], f32, tag="r0")
                    nc.vector.tensor_scalar_max(r0[:], u_t[:], 0.0)
                    nc.vector.tensor_tensor(out=e0[:], in0=e0[:], in1=r0[:], op=mybir.AluOpType.add)
                    nc.vector.tensor_scalar_add(e0[:], e0[:], -1.0)
                    # u (=e0) is the next layer's input; transpose it
                    ub = npool.tile([128, 256], bf16, tag="ub")
                    nc.vector.tensor_copy(out=ub[:], in_=e0[:])
                    uT_ps = pnode.tile([128, 2, 128], bf16, tag="nps")
                    for t2 in range(2):
                        nc.tensor.transpose(out=uT_ps[:, t2, :], in_=ub[:, t2 * 128:(t2 + 1) * 128],
                                            identity=ident[:])
                    uT = npool.tile([128, 2, 128], bf16, tag="uT")
                    nc.vector.tensor_copy(out=uT[:], in_=uT_ps[:])
                    if l == 0:
                        # h2 = u @ W2 (feature-major), then back to node-major
                        h_ps = pnode.tile([128, 2, 128], f32, tag="nps")
                        for mt in range(2):
                            for kt in range(2):
                                nc.tensor.matmul(out=h_ps[:, mt, :],
                                                 lhsT=w2_t[:, kt, mt * 128:(mt + 1) * 128],
                                                 rhs=uT[:, kt, :],
                                                 start=(kt == 0), stop=(kt == 1))
                        h_T = npool.tile([128, 2, 128], bf16, tag="hT")
                        nc.vector.tensor_copy(out=h_T[:], in_=h_ps[:])
                        hb_ps = pnode.tile([128, 2, 128], bf16, tag="nps")
                        for t2 in range(2):
                            nc.tensor.transpose(out=hb_ps[:, t2, :], in_=h_T[:, t2, :],
                                                identity=ident[:])
                        hb = npool.tile([128, 256], f32, tag="hb")
                        nc.vector.tensor_copy(out=hb[:], in_=hb_ps[:].rearrange("p a b -> p (a b)"))
                        emit_table_rows(1, h_T, hb[:], w)
                    else:
                        # layer-3 table: h part is u itself
                        emit_table_rows(2, uT, e0[:], w)
                else:
                    s_rec = npool.tile([128, 1], f32, tag="srec3")
                    nc.vector.tensor_scalar_add(s_rec[:], agg_ps[:, 256:257], 1e-16)
                    nc.vector.reciprocal(out=s_rec[:], in_=s_rec[:])
                    u_t = npool.tile([128, 256], f32, tag="ut")
                    nc.vector.tensor_tensor(
                        out=u_t[:], in0=agg_ps[:, 0:256],
                        in1=s_rec[:].to_broadcast([128, 256]), op=mybir.AluOpType.mult)
                    ub = npool.tile([128, 256], bf16, tag="ub")
                    nc.vector.tensor_copy(out=ub[:], in_=u_t[:])
                    uT_ps = pnode.tile([128, 2, 128], bf16, tag="nps")
                    for t2 in range(2):
                        nc.tensor.transpose(out=uT_ps[:, t2, :], in_=ub[:, t2 * 128:(t2 + 1) * 128],
                                            identity=ident[:])
                    uT = npool.tile([128, 2, 128], bf16, tag="uT")
                    nc.vector.tensor_copy(out=uT[:], in_=uT_ps[:])
                    o_ps = pnode.tile([N_CLASSES, 128], f32, tag="nps")
                    for kt in range(2):
                        nc.tensor.matmul(out=o_ps[:], lhsT=wo_t[:, kt, :],
                                         rhs=uT[:, kt, :], start=(kt == 0), stop=(kt == 1))
                    ob = npool.tile([128, 128], bf16, tag="ob")
                    nc.vector.memset(ob[:], 0.0)
                    nc.vector.tensor_copy(out=ob[0:N_CLASSES, :], in_=o_ps[:])
                    on_ps = pnode.tile([128, 128], bf16, tag="nps")
                    nc.tensor.transpose(out=on_ps[:], in_=ob[:], identity=ident[:])
                    o_n = npool.tile([128, N_CLASSES], f32, tag="on")
                    nc.vector.tensor_copy(out=o_n[:], in_=on_ps[:, 0:N_CLASSES])
                    mx = npool.tile([128, 1], f32, tag="mx")
                    nc.vector.tensor_reduce(out=mx[:], in_=o_n[:], axis=mybir.AxisListType.X,
                                            op=mybir.AluOpType.max)
                    nc.vector.tensor_tensor(out=o_n[:], in0=o_n[:],
                                            in1=mx[:].to_broadcast([128, N_CLASSES]),
                                            op=mybir.AluOpType.subtract)
                    ex_t = npool.tile([128, N_CLASSES], f32, tag="ext")
                    nc.scalar.activation(out=ex_t[:], in_=o_n[:], func=EXPF)
                    sm = npool.tile([128, 1], f32, tag="sm")
                    nc.vector.tensor_reduce(out=sm[:], in_=ex_t[:], axis=mybir.AxisListType.X,
                                            op=mybir.AluOpType.add)
                    ln_t = npool.tile([128, 1], f32, tag="lnt")
                    nc.scalar.activation(out=ln_t[:], in_=sm[:], func=mybir.ActivationFunctionType.Ln)
                    res = npool.tile([128, N_CLASSES], f32, tag="res")
                    nc.vector.tensor_tensor(out=res[:], in0=o_n[:],
                                            in1=ln_t[:].to_broadcast([128, N_CLASSES]),
                                            op=mybir.AluOpType.subtract)
                    nc.sync.dma_start(out=outy[w * 128:(w + 1) * 128, :], in_=res[:])

                # half-wise partial AllGather overlapped with later windows
                if l < 2 and w in (2 * QT - 1, W - 1):
                    h = 0 if w == 2 * QT - 1 else 1
                    nc.gpsimd.collective_compute(
                        "AllGather", mybir.AluOpType.bypass,
                        replica_groups=[list(range(CORES))],
                        ins=[Tsl_h[h].opt()],
                        outs=[Tnext[h * HBLK:(h + 1) * HBLK, :]])

    nc.compile()
    return nc


_CACHE = {}


def _make_in_maps(ins, per_core):
    x = np.asarray(ins["x"], np.float32)
    W1n = np.asarray(ins["W1"], np.float32)
    W2n = np.asarray(ins["W2"], np.float32)
    Won = np.asarray(ins["Wo"], np.float32)
    al1n, ar1n = np.asarray(ins["al1"], np.float32), np.asarray(ins["ar1"], np.float32)
    al2n, ar2n = np.asarray(ins["al2"], np.float32), np.asarray(ins["ar2"], np.float32)
    alon, aron = np.asarray(ins["alo"], np.float32), np.asarray(ins["aro"], np.float32)

    w1_h = W1n.astype(BF16)                                          # [128, 256]
    w2_h = np.ascontiguousarray(
        W2n.reshape(2, 128, 256).transpose(1, 0, 2)).astype(BF16)    # [128, 2, 256]
    wo_h = np.ascontiguousarray(
        Won.reshape(2, 128, N_CLASSES).transpose(1, 0, 2)).astype(BF16)
    alar = np.zeros((3, 256, 16), np.float32)
    alar[0] = _alar_block(al1n, ar1n, 256)
    alar[1] = _alar_block(al2n, ar2n, 256)
    alar[2][:, 0] = Won @ alon[0]
    alar[2][:, 8] = Won @ aron[0]
    alar_h = np.ascontiguousarray(
        alar.reshape(3, 2, 128, 16).transpose(2, 0, 1, 3)).astype(BF16)  # [128, 3, 2, 16]

    mneg = np.zeros((1, SP), np.float32)
    mneg[0, SPR:] = -30000.0
    mneg_h = mneg.astype(BF16)

    in_maps = []
    for c in range(CORES):
        xs = np.zeros((SP, IN_F), np.float32)
        xs[:SPR] = x[c * SPR:(c + 1) * SPR]
        pc = per_core[c]
        in_maps.append(dict(
            xsl=xs, gilo=pc["gilo"], gihi=pc["gihi"],
            selS=pc["selS"], selT=pc["selT"], mneg=mneg_h,
            w1=w1_h, w2=w2_h, wo=wo_h, alar=alar_h))
    return in_maps


def kernel(x, src, dst, W1, al1, ar1, W2, al2, ar2, Wo, alo, aro):
    src = np.asarray(src, np.int32)
    dst = np.asarray(dst, np.int32)

    per_core, CL, CH, CPW = _host_prep(src, dst)

    key = (CL, CH)
    if key not in _CACHE:
        _CACHE[key] = _build_program(CL, CH, CPW)
    nc = _CACHE[key]

    in_maps = _make_in_maps(
        dict(x=x, W1=W1, al1=al1, ar1=ar1, W2=W2, al2=al2, ar2=ar2,
             Wo=Wo, alo=alo, aro=aro), per_core)

    res = run_bass_kernel_spmd(nc, in_maps, list(range(CORES)))
    out = np.concatenate([res.results[c]["outy"][:SPR] for c in range(CORES)], axis=0)
    return out.astype(np.float32)



# revision 3
# speedup vs baseline: 32228.6150x; 32228.6150x over previous
"""3-layer GAT on Trainium2, 8 NeuronCores.

Strategy (dst-sharded, replicated tables):
- Nodes are remapped into 8 slices of 6656 rows (6250 real + pad); each core
  owns one slice of destination nodes and all edges pointing into it.
- Per layer, every core builds its slice of a node table
  [h(256) | Pl(8) | pl(8) | Pr(8) | pr(8) | pad] in bf16 (768B rows) where
  Pl=exp(el), pl=exp(0.2*el), Pr=exp(er), pr=exp(0.2*er); an AllGather
  replicates the full 53248-row table to every core.
- Edge phase per 128-dst window: dma_gather fetches the src rows for the
  window's edges (int16 indices; low/high table halves as two gathers),
  edge weights use exp(leaky_relu(el+er)) = max(Pl*Pr, pl*pr) (exact for
  slope<1), destination-side Pr/pr are expanded edge-wise with a one-hot
  matmul, and a second one-hot matmul segment-sums ex*h and ex into PSUM
  per dst. Softmax normalization happens after aggregation (out/s): exact
  because alpha = ex/sum(ex) is invariant to per-dst scaling (this also
  makes the explicit segment-max unnecessary; |logits| < 10 here).
- Layer 3 commutes the output projection with aggregation:
  sum(ex*h2) @ Wo == sum(ex*(h2@Wo)), with el3 = h2 @ (Wo@alo^T).
"""
import numpy as np
import ml_dtypes
from contextlib import ExitStack

import concourse.bass as bass
import concourse.tile as tile
from concourse import bacc, mybir
from concourse.bass_utils import run_bass_kernel_spmd
from concourse.masks import make_identity

BF16 = ml_dtypes.bfloat16

N_NODES = 50000
IN_F = 128
N_CLASSES = 40
CORES = 8
SPR = 6250          # real dst nodes per core
SP = 6656           # slice rows per core (52 * 128)
NT = SP * CORES     # 53248 padded table rows
HALF = 32768        # int16-addressable table half
W = 49              # dst windows per core (ceil(6250/128))
ROW = 384           # table row elems (bf16 -> 768B, multiple of 256B)
NTILE = SP // 128   # 52 node tiles per slice
EXPF = mybir.ActivationFunctionType.Exp


QT = NTILE // 4      # 13 tiles per collective quarter
QROWS = QT * 128     # 1664 slice rows per quarter
QBLK = QROWS * CORES # 13312 table rows per quarter


HT = NTILE // 2      # 26 tiles per collective half
HROWS = HT * 128     # 3328 slice rows per half
HBLK = HROWS * CORES # 26624 table rows per half


def _remap(n):
    """Global table row for node n, laid out (half, core, tile, row) so that
    half-wise partial AllGathers are contiguous in both the slice and the
    full table (AllGather concatenates per-core inputs)."""
    c = n // SPR
    r = n % SPR
    t = r // 128
    h = t // HT
    return h * HBLK + c * HROWS + (t % HT) * 128 + (r % 128)


def _wrap16(vals, nidx):
    """dma_gather index layout: flat idx i -> [i%16, i//16], replicated to
    all 8 groups of 16 partitions."""
    blk = np.zeros((16, nidx // 16), np.int16)
    blk[np.arange(nidx) % 16, np.arange(nidx) // 16] = vals
    return np.tile(blk, (8, 1))


def _host_prep(src, dst):
    gsrc = _remap(src.astype(np.int64))
    d64 = dst.astype(np.int64)
    core = d64 // SPR
    ld = d64 % SPR
    w = ld >> 7
    dstl = (ld & 127).astype(np.int64)
    ishigh = (gsrc >= HALF).astype(np.int64)
    wg = core * W + w

    order = np.lexsort((ishigh, wg))
    gsrc_s, dstl_s, wg_s, hi_s = gsrc[order], dstl[order], wg[order], ishigh[order]

    gkey = wg_s * 2 + hi_s
    uniq, starts, counts = np.unique(gkey, return_index=True, return_counts=True)
    pos_in_grp = np.arange(len(gkey)) - np.repeat(starts, counts)

    nlow = np.zeros(CORES * W, np.int64)
    nhigh = np.zeros(CORES * W, np.int64)
    for u, c in zip(uniq, counts):
        (nlow if u % 2 == 0 else nhigh)[u // 2] = c
    CL = int(np.ceil(nlow.max() / 128))
    CH = int(np.ceil(nhigh.max() / 128))
    CPW = CL + CH

    w_local = wg_s % W
    slot = w_local * (CPW * 128) + np.where(hi_s == 1, CL * 128, 0) + pos_in_grp
    core_s = wg_s // W

    NSLOT = W * CPW * 128
    per_core = []
    for c in range(CORES):
        m = core_s == c
        sl = slot[m]
        gs = gsrc_s[m]
        dl = dstl_s[m]
        hi = hi_s[m]

        lowidx = np.zeros(W * CL * 128, np.int16)
        highidx = np.zeros(W * CH * 128, np.int16)
        wl = sl // (CPW * 128)
        ks = sl % (CPW * 128)
        lm = hi == 0
        lowidx[wl[lm] * (CL * 128) + ks[lm]] = gs[lm].astype(np.int16)
        hm = hi == 1
        highidx[wl[hm] * (CH * 128) + (ks[hm] - CL * 128)] = (gs[hm] - HALF).astype(np.int16)

        gilo = np.concatenate(
            [_wrap16(lowidx[i * CL * 128:(i + 1) * CL * 128], CL * 128) for i in range(W)], axis=1)
        gihi = np.concatenate(
            [_wrap16(highidx[i * CH * 128:(i + 1) * CH * 128], CH * 128) for i in range(W)], axis=1)

        # one-hot streams; pad slots keep all-zero column/row -> zero contribution
        selS = np.zeros((128, NSLOT), BF16)    # sel[d, slot]
        selT = np.zeros((128, NSLOT), BF16)    # selT[e_lane, chunk*128 + d]
        e_lane = sl % 128
        chunk = sl // 128
        selS[dl, sl] = 1
        selT[e_lane, chunk * 128 + dl] = 1

        per_core.append(dict(gilo=gilo, gihi=gihi, selS=selS, selT=selT))
    return per_core, CL, CH, CPW


def _alar_block(al, ar, fout):
    """[fout, 16]: col j (<8) extracts el head j, col j+8 er head j."""
    H, F = al.shape
    m = np.zeros((fout, 16), np.float32)
    for j in range(H):
        m[j * F:(j + 1) * F, j] = al[j]
        m[j * F:(j + 1) * F, j + 8] = ar[j]
    return m


def _build_program(CL, CH, CPW):
    nc = bacc.Bacc("TRN2", target_bir_lowering=False, debug=False, num_devices=CORES)
    f32, bf16, i16 = mybir.dt.float32, mybir.dt.bfloat16, mybir.dt.int16

    xsl = nc.declare_dram_parameter("xsl", [SP, IN_F], f32, isOutput=False)
    gilo_d = nc.declare_dram_parameter("gilo", [128, W * CL * 8], i16, isOutput=False)
    gihi_d = nc.declare_dram_parameter("gihi", [128, W * CH * 8], i16, isOutput=False)
    selS_d = nc.declare_dram_parameter("selS", [128, W * CPW * 128], bf16, isOutput=False)
    selT_d = nc.declare_dram_parameter("selT", [128, W * CPW * 128], bf16, isOutput=False)
    mneg_d = nc.declare_dram_parameter("mneg", [1, SP], bf16, isOutput=False)
    w1_d = nc.declare_dram_parameter("w1", [IN_F, 256], bf16, isOutput=False)
    w2_d = nc.declare_dram_parameter("w2", [128, 2, 256], bf16, isOutput=False)
    wo_d = nc.declare_dram_parameter("wo", [128, 2, N_CLASSES], bf16, isOutput=False)
    alar_d = nc.declare_dram_parameter("alar", [128, 3, 2, 16], bf16, isOutput=False)
    outy = nc.declare_dram_parameter("outy", [SP, N_CLASSES], f32, isOutput=True)

    with ExitStack() as ctx:
        tc = ctx.enter_context(tile.TileContext(nc))
        const = ctx.enter_context(tc.tile_pool(name="const", bufs=1))
        dram = ctx.enter_context(tc.tile_pool(name="dram", bufs=1, space="DRAM"))
        gpool = ctx.enter_context(tc.tile_pool(name="gpool", bufs=2))
        spool = ctx.enter_context(tc.tile_pool(name="spool", bufs=2))
        npool = ctx.enter_context(tc.tile_pool(name="npool", bufs=2))
        pwin = ctx.enter_context(tc.tile_pool(name="pwin", bufs=2, space="PSUM"))
        pnode = ctx.enter_context(tc.tile_pool(name="pnode", bufs=2, space="PSUM"))

        Tsl_h = [dram.tile([HROWS, ROW], bf16, name="tsl0"),
                 dram.tile([HROWS, ROW], bf16, name="tsl1")]
        TfullA = dram.tile([NT, ROW], bf16)
        TfullB = dram.tile([NT, ROW], bf16)

        def Tslice_rows(r0, r1):
            h = r0 // HROWS
            assert (r1 - 1) // HROWS == h
            return Tsl_h[h][r0 - h * HROWS:r1 - h * HROWS, :]

        gilo_t = const.tile([128, W * CL * 8], i16)
        nc.sync.dma_start(out=gilo_t[:], in_=gilo_d[:, :])
        gihi_t = const.tile([128, W * CH * 8], i16)
        nc.sync.dma_start(out=gihi_t[:], in_=gihi_d[:, :])
        mneg_t = const.tile([1, SP], bf16)
        nc.sync.dma_start(out=mneg_t[:], in_=mneg_d[:, :])
        w1_t = const.tile([IN_F, 256], bf16)
        nc.sync.dma_start(out=w1_t[:], in_=w1_d[:, :])
        w2_t = const.tile([128, 2, 256], bf16)
        nc.sync.dma_start(out=w2_t[:], in_=w2_d[:, :, :])
        wo_t = const.tile([128, 2, N_CLASSES], bf16)
        nc.sync.dma_start(out=wo_t[:], in_=wo_d[:, :, :])
        alar_t = const.tile([128, 3, 2, 16], bf16)
        nc.sync.dma_start(out=alar_t[:], in_=alar_d[:, :, :, :])
        ident = const.tile([128, 128], bf16)
        make_identity(nc, ident[:])
        ones16 = const.tile([1, 16], bf16)
        nc.vector.memset(ones16[:], 1.0)

        def emit_table_rows(al_idx, h_T, h_node_src, w):
            """Assemble table row tile [h|Pl|pl|Pr|pr|0] for node rows
            [w*128,(w+1)*128) and DMA it into Tslice.
            al_idx: which alar block; h_T: [128,2,128] bf16 feature-major;
            h_node_src: AP with node-major h values [128, 256] (any float)."""
            row_t = npool.tile([128, ROW], bf16, tag="row")
            nc.vector.tensor_copy(out=row_t[:, 0:256], in_=h_node_src)
            el_ps = pnode.tile([16, 128], f32, tag="nps")
            for kt in range(2):
                nc.tensor.matmul(out=el_ps[:], lhsT=alar_t[:, al_idx, kt, :],
                                 rhs=h_T[:, kt, :], start=(kt == 0), stop=False)
            nc.tensor.matmul(out=el_ps[:], lhsT=ones16[:],
                             rhs=mneg_t[:, w * 128:(w + 1) * 128], start=False, stop=True)
            P_t = npool.tile([128, 128], bf16, tag="Pt")
            p_t = npool.tile([128, 128], bf16, tag="pt")
            nc.scalar.activation(out=P_t[0:16, :], in_=el_ps[:], func=EXPF, scale=1.0)
            nc.scalar.activation(out=p_t[0:16, :], in_=el_ps[:], func=EXPF, scale=0.2)
            Pt_ps = pnode.tile([128, 128], bf16, tag="nps")
            nc.tensor.transpose(out=Pt_ps[:], in_=P_t[:], identity=ident[:])
            nc.vector.tensor_copy(out=row_t[:, 256:264], in_=Pt_ps[:, 0:8])     # Pl
            nc.vector.tensor_copy(out=row_t[:, 272:280], in_=Pt_ps[:, 8:16])    # Pr
            pt_ps = pnode.tile([128, 128], bf16, tag="nps")
            nc.tensor.transpose(out=pt_ps[:], in_=p_t[:], identity=ident[:])
            nc.vector.tensor_copy(out=row_t[:, 264:272], in_=pt_ps[:, 0:8])     # pl
            nc.vector.tensor_copy(out=row_t[:, 280:288], in_=pt_ps[:, 8:16])    # pr
            nc.vector.memset(row_t[:, 288:ROW], 0.0)
            nc.sync.dma_start(out=Tslice_rows(w * 128, (w + 1) * 128), in_=row_t[:])

        # ---- P0: layer-1 table from x ----
        for t in range(NTILE):
            x_t = npool.tile([128, IN_F], f32, tag="xt")
            nc.sync.dma_start(out=x_t[:], in_=xsl[t * 128:(t + 1) * 128, :])
            xb = npool.tile([128, IN_F], bf16, tag="xb")
            nc.vector.tensor_copy(out=xb[:], in_=x_t[:])
            xT_ps = pnode.tile([128, 128], bf16, tag="nps")
            nc.tensor.transpose(out=xT_ps[:], in_=xb[:], identity=ident[:])
            xT = npool.tile([128, 128], bf16, tag="xT")
            nc.vector.tensor_copy(out=xT[:], in_=xT_ps[:])
            h_ps = pnode.tile([128, 2, 128], f32, tag="nps")
            for mt in range(2):
                nc.tensor.matmul(out=h_ps[:, mt, :], lhsT=w1_t[:, mt * 128:(mt + 1) * 128],
                                 rhs=xT[:], start=True, stop=True)
            h_T = npool.tile([128, 2, 128], bf16, tag="hT")
            nc.vector.tensor_copy(out=h_T[:], in_=h_ps[:])
            hb_ps = pnode.tile([128, 2, 128], bf16, tag="nps")
            for t2 in range(2):
                nc.tensor.transpose(out=hb_ps[:, t2, :], in_=h_T[:, t2, :], identity=ident[:])
            hb = npool.tile([128, 256], f32, tag="hb")
            nc.vector.tensor_copy(out=hb[:], in_=hb_ps[:].rearrange("p a b -> p (a b)"))
            emit_table_rows(0, h_T, hb[:], t)
            if t in (2 * QT - 1, NTILE - 1):
                h = 0 if t == 2 * QT - 1 else 1
                nc.gpsimd.collective_compute(
                    "AllGather", mybir.AluOpType.bypass,
                    replica_groups=[list(range(CORES))],
                    ins=[Tsl_h[h].opt()],
                    outs=[TfullA[h * HBLK:(h + 1) * HBLK, :]])

        # ---- 3 layers of windowed edge aggregation ----
        for l in range(3):
            Tf = TfullA if l % 2 == 0 else TfullB
            Tnext = TfullB if l % 2 == 0 else TfullA
            for w in range(W):
                g_win = gpool.tile([128, CPW, ROW], bf16, tag="gwin")
                nc.gpsimd.dma_gather(
                    out_ap=g_win[:, 0:CL, :], in_ap=Tf[0:HALF, :],
                    idxs_ap=gilo_t[:, w * CL * 8:(w + 1) * CL * 8],
                    num_idxs=CL * 128, num_idxs_reg=CL * 128, elem_size=ROW,
                    single_packet=False)
                nc.gpsimd.dma_gather(
                    out_ap=g_win[:, CL:CPW, :], in_ap=Tf[HALF:NT, :],
                    idxs_ap=gihi_t[:, w * CH * 8:(w + 1) * CH * 8],
                    num_idxs=CH * 128, num_idxs_reg=CH * 128, elem_size=ROW,
                    single_packet=False)
                selw = spool.tile([128, CPW, 128], bf16, tag="selw")
                nc.sync.dma_start(out=selw[:], in_=selS_d[:, w * CPW * 128:(w + 1) * CPW * 128])
                selTw = spool.tile([128, CPW, 128], bf16, tag="selTw")
                nc.sync.dma_start(out=selTw[:], in_=selT_d[:, w * CPW * 128:(w + 1) * CPW * 128])
                prpr = spool.tile([128, 16], bf16, tag="prpr")
                nc.sync.dma_start(out=prpr[:], in_=Tslice_rows(w * 128, (w + 1) * 128)[:, 272:288])

                # expand dst-side Pr/pr to edge-major: pp[e, 16]
                pp_ps = pwin.tile([128, CPW, 16], f32, tag="ppps")
                for c in range(CPW):
                    nc.tensor.matmul(out=pp_ps[:, c, :], lhsT=selw[:, c, :],
                                     rhs=prpr[:], start=True, stop=True)
                # ex = max(Pl*Pr, pl*pr)
                cand = spool.tile([128, CPW, 2, 8], bf16, tag="cand")
                nc.vector.tensor_tensor(
                    out=cand[:],
                    in0=g_win[:, :, 256:272].rearrange("p c (a h) -> p c a h", a=2),
                    in1=pp_ps[:].rearrange("p c (a h) -> p c a h", a=2),
                    op=mybir.AluOpType.mult)
                rhs_w = spool.tile([128, CPW, 264], bf16, tag="rhsw")
                nc.vector.tensor_tensor(
                    out=rhs_w[:, :, 256:264], in0=cand[:, :, 0, :], in1=cand[:, :, 1, :],
                    op=mybir.AluOpType.max)
                if l < 2:
                    nc.vector.tensor_tensor(
                        out=rhs_w[:, :, 0:256].rearrange("p c (h f) -> p c h f", h=8),
                        in0=g_win[:, :, 0:256].rearrange("p c (h f) -> p c h f", h=8),
                        in1=rhs_w[:, :, 256:264].rearrange("p c (h o) -> p c h o", o=1)
                            .to_broadcast([128, CPW, 8, 32]),
                        op=mybir.AluOpType.mult)
                else:
                    nc.vector.tensor_tensor(
                        out=rhs_w[:, :, 0:256],
                        in0=g_win[:, :, 0:256],
                        in1=rhs_w[:, :, 256:257].to_broadcast([128, CPW, 256]),
                        op=mybir.AluOpType.mult)
                agg_ps = pwin.tile([128, 264], f32, tag="aggps")
                for c in range(CPW):
                    nc.tensor.matmul(out=agg_ps[:], lhsT=selTw[:, c, :], rhs=rhs_w[:, c, :],
                                     start=(c == 0), stop=(c == CPW - 1))

                # ---- per-window node phase ----
                if l < 2:
                    s_rec = npool.tile([128, 8], f32, tag="srec")
                    nc.vector.tensor_scalar_add(s_rec[:], agg_ps[:, 256:264], 1e-16)
                    nc.vector.reciprocal(out=s_rec[:], in_=s_rec[:])
                    u_t = npool.tile([128, 256], f32, tag="ut")
                    nc.vector.tensor_tensor(
                        out=u_t[:].rearrange("p (h f) -> p h f", h=8),
                        in0=agg_ps[:, 0:256].rearrange("p (h f) -> p h f", h=8),
                        in1=s_rec[:].rearrange("p (h o) -> p h o", o=1).to_broadcast([128, 8, 32]),
                        op=mybir.AluOpType.mult)
                    # elu(x) = exp(min(x,0)) - 1 + relu(x)
                    m0 = npool.tile([128, 256], f32, tag="m0")
                    nc.vector.tensor_scalar_min(m0[:], u_t[:], 0.0)
                    e0 = npool.tile([128, 256], f32, tag="e0")
                    nc.scalar.activation(out=e0[:], in_=m0[:], func=EXPF)
                    r0 = npool.tile([128, 256], f32, tag="r0")
                    nc.vector.tensor_scalar_max(r0[:], u_t[:], 0.0)
                    nc.vector.tensor_tensor(out=e0[:], in0=e0[:], in1=r0[:], op=mybir.AluOpType.add)
                    nc.vector.tensor_scalar_add(e0[:], e0[:], -1.0)
                    # u (=e0) is the next layer's input; transpose it
                    ub = npool.tile([128, 256], bf16, tag="ub")
                    nc.vector.tensor_copy(out=ub[:], in_=e0[:])
                    uT_ps = pnode.tile([128, 2, 128], bf16, tag="nps")
                    for t2 in range(2):
                        nc.tensor.transpose(out=uT_ps[:, t2, :], in_=ub[:, t2 * 128:(t2 + 1) * 128],
                                            identity=ident[:])
                    uT = npool.tile([128, 2, 128], bf16, tag="uT")
                    nc.vector.tensor_copy(out=uT[:], in_=uT_ps[:])
                    if l == 0:
                        # h2 = u @ W2 (feature-major), then back to node-major
                        h_ps = pnode.tile([128, 2, 128], f32, tag="nps")
                        for mt in range(2):
                            for kt in range(2):
                                nc.tensor.matmul(out=h_ps[:, mt, :],
                                                 lhsT=w2_t[:, kt, mt * 128:(mt + 1) * 128],
                                                 rhs=uT[:, kt, :],
                                                 start=(kt == 0), stop=(kt == 1))
                        h_T = npool.tile([128, 2, 128], bf16, tag="hT")
                        nc.vector.tensor_copy(out=h_T[:], in_=h_ps[:])
                        hb_ps = pnode.tile([128, 2, 128], bf16, tag="nps")
                        for t2 in range(2):
                            nc.tensor.transpose(out=hb_ps[:, t2, :], in_=h_T[:, t2, :],
                                                identity=ident[:])
                        hb = npool.tile([128, 256], f32, tag="hb")
                        nc.vector.tensor_copy(out=hb[:], in_=hb_ps[:].rearrange("p a b -> p (a b)"))
                        emit_table_rows(1, h_T, hb[:], w)
                    else:
                        # layer-3 table: h part is u itself
                        emit_table_rows(2, uT, e0[:], w)
                else:
                    s_rec = npool.tile([128, 1], f32, tag="srec3")
                    nc.vector.tensor_scalar_add(s_rec[:], agg_ps[:, 256:257], 1e-16)
                    nc.vector.reciprocal(out=s_rec[:], in_=s_rec[:])
                    u_t = npool.tile([128, 256], f32, tag="ut")
                    nc.vector.tensor_tensor(
                        out=u_t[:], in0=agg_ps[:, 0:256],
                        in1=s_rec[:].to_broadcast([128, 256]), op=mybir.AluOpType.mult)
                    ub = npool.tile([128, 256], bf16, tag="ub")
                    nc.vector.tensor_copy(out=ub[:], in_=u_t[:])
                    uT_ps = pnode.tile([128, 2, 128], bf16, tag="nps")
                    for t2 in range(2):
                        nc.tensor.transpose(out=uT_ps[:, t2, :], in_=ub[:, t2 * 128:(t2 + 1) * 128],
                                            identity=ident[:])
                    uT = npool.tile([128, 2, 128], bf16, tag="uT")
                    nc.vector.tensor_copy(out=uT[:], in_=uT_ps[:])
                    o_ps = pnode.tile([N_CLASSES, 128], f32, tag="nps")
                    for kt in range(2):
                        nc.tensor.matmul(out=o_ps[:], lhsT=wo_t[:, kt, :],
                                         rhs=uT[:, kt, :], start=(kt == 0), stop=(kt == 1))
                    ob = npool.tile([128, 128], bf16, tag="ob")
                    nc.vector.memset(ob[:], 0.0)
                    nc.vector.tensor_copy(out=ob[0:N_CLASSES, :], in_=o_ps[:])
                    on_ps = pnode.tile([128, 128], bf16, tag="nps")
                    nc.tensor.transpose(out=on_ps[:], in_=ob[:], identity=ident[:])
                    o_n = npool.tile([128, N_CLASSES], f32, tag="on")
                    nc.vector.tensor_copy(out=o_n[:], in_=on_ps[:, 0:N_CLASSES])
                    mx = npool.tile([128, 1], f32, tag="mx")
                    nc.vector.tensor_reduce(out=mx[:], in_=o_n[:], axis=mybir.AxisListType.X,
                                            op=mybir.AluOpType.max)
                    nc.vector.tensor_tensor(out=o_n[:], in0=o_n[:],
                                            in1=mx[:].to_broadcast([128, N_CLASSES]),
                                            op=mybir.AluOpType.subtract)
                    ex_t = npool.tile([128, N_CLASSES], f32, tag="ext")
                    nc.scalar.activation(out=ex_t[:], in_=o_n[:], func=EXPF)
                    sm = npool.tile([128, 1], f32, tag="sm")
                    nc.vector.tensor_reduce(out=sm[:], in_=ex_t[:], axis=mybir.AxisListType.X,
                                            op=mybir.AluOpType.add)
                    ln_t = npool.tile([128, 1], f32, tag="lnt")
                    nc.scalar.activation(out=ln_t[:], in_=sm[:], func=mybir.ActivationFunctionType.Ln)
                    res = npool.tile([128, N_CLASSES], f32, tag="res")
                    nc.vector.tensor_tensor(out=res[:], in0=o_n[:],
                                            in1=ln_t[:].to_broadcast([128, N_CLASSES]),
                                            op=mybir.AluOpType.subtract)
                    nc.sync.dma_start(out=outy[w * 128:(w + 1) * 128, :], in_=res[:])

                # half-wise partial AllGather overlapped with later windows
                if l < 2 and w in (2 * QT - 1, W - 1):
                    h = 0 if w == 2 * QT - 1 else 1
                    nc.gpsimd.collective_compute(
                        "AllGather", mybir.AluOpType.bypass,
                        replica_groups=[list(range(CORES))],
                        ins=[Tsl_h[h].opt()],
                        outs=[Tnext[h * HBLK:(h + 1) * HBLK, :]])

    nc.compile()
    return nc


_CACHE = {}
_LAST = {}


def _make_in_maps(ins, per_core):
    x = np.asarray(ins["x"], np.float32)
    W1n = np.asarray(ins["W1"], np.float32)
    W2n = np.asarray(ins["W2"], np.float32)
    Won = np.asarray(ins["Wo"], np.float32)
    al1n, ar1n = np.asarray(ins["al1"], np.float32), np.asarray(ins["ar1"], np.float32)
    al2n, ar2n = np.asarray(ins["al2"], np.float32), np.asarray(ins["ar2"], np.float32)
    alon, aron = np.asarray(ins["alo"], np.float32), np.asarray(ins["aro"], np.float32)

    w1_h = W1n.astype(BF16)                                          # [128, 256]
    w2_h = np.ascontiguousarray(
        W2n.reshape(2, 128, 256).transpose(1, 0, 2)).astype(BF16)    # [128, 2, 256]
    wo_h = np.ascontiguousarray(
        Won.reshape(2, 128, N_CLASSES).transpose(1, 0, 2)).astype(BF16)
    alar = np.zeros((3, 256, 16), np.float32)
    alar[0] = _alar_block(al1n, ar1n, 256)
    alar[1] = _alar_block(al2n, ar2n, 256)
    alar[2][:, 0] = Won @ alon[0]
    alar[2][:, 8] = Won @ aron[0]
    alar_h = np.ascontiguousarray(
        alar.reshape(3, 2, 128, 16).transpose(2, 0, 1, 3)).astype(BF16)  # [128, 3, 2, 16]

    mneg = np.zeros((1, SP), np.float32)
    mneg[0, SPR:] = -30000.0
    mneg_h = mneg.astype(BF16)

    in_maps = []
    for c in range(CORES):
        xs = np.zeros((SP, IN_F), np.float32)
        xs[:SPR] = x[c * SPR:(c + 1) * SPR]
        pc = per_core[c]
        in_maps.append(dict(
            xsl=xs, gilo=pc["gilo"], gihi=pc["gihi"],
            selS=pc["selS"], selT=pc["selT"], mneg=mneg_h,
            w1=w1_h, w2=w2_h, wo=wo_h, alar=alar_h))
    return in_maps


def kernel(x, src, dst, W1, al1, ar1, W2, al2, ar2, Wo, alo, aro):
    src = np.asarray(src, np.int32)
    dst = np.asarray(dst, np.int32)

    per_core, CL, CH, CPW = _host_prep(src, dst)

    key = (CL, CH)
    if key not in _CACHE:
        _CACHE[key] = _build_program(CL, CH, CPW)
    nc = _CACHE[key]

    in_maps = _make_in_maps(
        dict(x=x, W1=W1, al1=al1, ar1=ar1, W2=W2, al2=al2, ar2=ar2,
             Wo=Wo, alo=alo, aro=aro), per_core)

    _LAST["nc"] = nc
    _LAST["in_maps"] = in_maps
    res = run_bass_kernel_spmd(nc, in_maps, list(range(CORES)))
    out = np.concatenate([res.results[c]["outy"][:SPR] for c in range(CORES)], axis=0)
    return out.astype(np.float32)



# revision 14
# speedup vs baseline: 33404.6567x; 1.0365x over previous
"""3-layer GAT on Trainium2, 8 NeuronCores — v2.

Strategy (dst-sharded, replicated tables). The hard resource is the GpSimd
Q7 cluster: every dma_gather descriptor costs ~8-10ns of Q7 descriptor
generation, and the kernel needs one gathered table row per edge per layer
(~100k rows/core/layer). v2 therefore (a) trims pad slots via the Q7
kernel's trailing-negative-index trim, (b) keeps Q7 busy through layer
boundaries: tables AllGather in QUARTERS, windows are processed in
reversed-quarter order so early quarters of the next table finish mid-layer,
and each window's high-half gather (table rows >= 26624) runs G windows
ahead of its low-half gather, bridging the table-ready gap.

Table row (768B, bf16): [h(256, (f,h)-interleaved) | el(8) | er(8) | pad96].
Edge weights: ex = exp(leaky_relu(el_src + er_dst)) computed exactly as
max(exp(e), exp(0.2 e)). Softmax normalization happens after aggregation
(alpha = ex/sum(ex) is scale-invariant per dst; |logits| < 10 so no
segment-max needed). Layer 3 commutes the output projection with
aggregation: sum(ex*h2) @ Wo == sum(ex*(h2@Wo)).
"""
import numpy as np
import ml_dtypes
from collections import deque
from contextlib import ExitStack

import concourse.bass as bass
import concourse.tile as tile
from concourse import bacc, mybir
from concourse.bass_utils import run_bass_kernel_spmd
from concourse.masks import make_identity

BF16 = ml_dtypes.bfloat16

N_NODES = 50000
IN_F = 128
N_CLASSES = 40
CORES = 8
SPR = 6250          # real dst nodes per core
SP = 6656           # slice rows per core (52 * 128)
NT = SP * CORES     # 53248 padded table rows
W = 49              # dst windows per core (ceil(6250/128))
ROW = 384           # table row elems (bf16 -> 768B, multiple of 256B)
NTILE = SP // 128   # 52 node tiles per slice
NEG_SLOPE = 0.2
EXPF = mybir.ActivationFunctionType.Exp
COPYF = mybir.ActivationFunctionType.Copy
LNF = mybir.ActivationFunctionType.Ln

QT = NTILE // 4      # 13 tiles per table quarter
QROWS = QT * 128     # 1664 slice rows per quarter
QBLK = QROWS * CORES # 13312 table rows per quarter
HALF = 2 * QBLK      # 26624: gather low/high split == quarters {0,1}/{2,3}
G = 5                # high-gather lookahead (windows)

# reversed-quarter processing order: quarter q3's windows first, so the next
# table's quarters complete (and AllGather) as early as possible.
WORDER = list(range(39, 49)) + list(range(26, 39)) + list(range(13, 26)) + list(range(0, 13))
TORDER = list(range(39, 52)) + list(range(26, 39)) + list(range(13, 26)) + list(range(0, 13))

# h column interleave: new col k holds old col (k%8)*32 + (k//8), i.e.
# feature-major with the 8 heads contiguous. Lets per-head edge weights
# broadcast with a unit-stride inner dim of 8 on the DVE.
PERM = np.array([(k % 8) * 32 + (k // 8) for k in range(256)], np.int64)


def _remap(n):
    """Global table row for node n, laid out (quarter, core, tile, row) so
    quarter-wise partial AllGathers are contiguous in slice and table."""
    c = n // SPR
    r = n % SPR
    t = r // 128
    q = t // QT
    return q * QBLK + c * QROWS + (t % QT) * 128 + (r % 128)


def _wrap16(vals, nidx):
    """dma_gather index layout: flat idx i -> [i%16, i//16], replicated to
    all 8 groups of 16 partitions."""
    blk = np.zeros((16, nidx // 16), np.int16)
    blk[np.arange(nidx) % 16, np.arange(nidx) // 16] = vals
    return np.tile(blk, (8, 1))


def _host_prep(src, dst):
    gsrc = _remap(src.astype(np.int64))
    d64 = dst.astype(np.int64)
    core = d64 // SPR
    ld = d64 % SPR
    w = ld >> 7
    dstl = (ld & 127).astype(np.int64)
    ishigh = (gsrc >= HALF).astype(np.int64)
    wg = core * W + w

    order = np.lexsort((ishigh, wg))
    gsrc_s, dstl_s, wg_s, hi_s = gsrc[order], dstl[order], wg[order], ishigh[order]

    gkey = wg_s * 2 + hi_s
    uniq, starts, counts = np.unique(gkey, return_index=True, return_counts=True)
    pos_in_grp = np.arange(len(gkey)) - np.repeat(starts, counts)

    nlow = np.zeros(CORES * W, np.int64)
    nhigh = np.zeros(CORES * W, np.int64)
    for u, c in zip(uniq, counts):
        (nlow if u % 2 == 0 else nhigh)[u // 2] = c
    CL = int(np.ceil(nlow.max() / 128))
    CH = int(np.ceil(nhigh.max() / 128))
    CPW = CL + CH
    # Per-window descriptor budget: the Q7 gather kernel trims trailing
    # negative idxs, but decode reserves ring space from num_idxs_reg, so the
    # trim point must be IDENTICAL on every core (the program is shared).
    # KL[w] = chunks needed by the worst core; pads below that boundary stay
    # 0 (valid row, zero sel weight), above it -1 (trimmed, never fetched).
    KL = [max(1, int(np.ceil(max(nlow[c * W + w2] for c in range(CORES)) / 128)))
          for w2 in range(W)]
    KH = [max(1, int(np.ceil(max(nhigh[c * W + w2] for c in range(CORES)) / 128)))
          for w2 in range(W)]

    w_local = wg_s % W
    slot = w_local * (CPW * 128) + np.where(hi_s == 1, CL * 128, 0) + pos_in_grp
    core_s = wg_s // W

    NSLOT = W * CPW * 128
    per_core = []
    for c in range(CORES):
        m = core_s == c
        sl = slot[m]
        gs = gsrc_s[m]
        dl = dstl_s[m]
        hi = hi_s[m]

        lowidx = np.full(W * CL * 128, -1, np.int16)
        highidx = np.full(W * CH * 128, -1, np.int16)
        for w2 in range(W):
            lowidx[w2 * CL * 128:w2 * CL * 128 + KL[w2] * 128] = 0
            highidx[w2 * CH * 128:w2 * CH * 128 + KH[w2] * 128] = 0
        wl = sl // (CPW * 128)
        ks = sl % (CPW * 128)
        lm = hi == 0
        lowidx[wl[lm] * (CL * 128) + ks[lm]] = gs[lm].astype(np.int16)
        hm = hi == 1
        highidx[wl[hm] * (CH * 128) + (ks[hm] - CL * 128)] = (gs[hm] - HALF).astype(np.int16)

        gilo = np.concatenate(
            [_wrap16(lowidx[i * CL * 128:(i + 1) * CL * 128], CL * 128) for i in range(W)], axis=1)
        gihi = np.concatenate(
            [_wrap16(highidx[i * CH * 128:(i + 1) * CH * 128], CH * 128) for i in range(W)], axis=1)

        # one-hot streams; pad slots keep all-zero column/row -> zero contribution
        selS = np.zeros((128, NSLOT), BF16)    # sel[d, slot]
        selT = np.zeros((128, NSLOT), BF16)    # selT[e_lane, chunk*128 + d]
        e_lane = sl % 128
        chunk = sl // 128
        selS[dl, sl] = 1
        selT[e_lane, chunk * 128 + dl] = 1

        per_core.append(dict(gilo=gilo, gihi=gihi, selS=selS, selT=selT))
    return per_core, CL, CH, CPW, KL, KH


def _alar_block(al, ar, fout):
    """[fout, 16]: col j (<8) extracts el head j, col j+8 er head j."""
    H, F = al.shape
    m = np.zeros((fout, 16), np.float32)
    for j in range(H):
        m[j * F:(j + 1) * F, j] = al[j]
        m[j * F:(j + 1) * F, j + 8] = ar[j]
    return m


def _build_program(CL, CH, CPW, KL, KH):
    nc = bacc.Bacc("TRN2", target_bir_lowering=False, debug=False, num_devices=CORES)
    f32, bf16, i16 = mybir.dt.float32, mybir.dt.bfloat16, mybir.dt.int16

    xT_d = nc.declare_dram_parameter("xT", [IN_F, SP], bf16, isOutput=False)
    gilo_d = nc.declare_dram_parameter("gilo", [128, W * CL * 8], i16, isOutput=False)
    gihi_d = nc.declare_dram_parameter("gihi", [128, W * CH * 8], i16, isOutput=False)
    selS_d = nc.declare_dram_parameter("selS", [128, W * CPW * 128], bf16, isOutput=False)
    selT_d = nc.declare_dram_parameter("selT", [128, W * CPW * 128], bf16, isOutput=False)
    w1_d = nc.declare_dram_parameter("w1", [IN_F, 256], bf16, isOutput=False)
    w2_d = nc.declare_dram_parameter("w2", [128, 2, 256], bf16, isOutput=False)
    wo_d = nc.declare_dram_parameter("wo", [128, 2, N_CLASSES], bf16, isOutput=False)
    alar_d = nc.declare_dram_parameter("alar", [128, 3, 2, 16], bf16, isOutput=False)
    outy = nc.declare_dram_parameter("outy", [SP, N_CLASSES], f32, isOutput=True)

    with ExitStack() as ctx:
        tc = ctx.enter_context(tile.TileContext(nc))
        const = ctx.enter_context(tc.tile_pool(name="const", bufs=1))
        dram = ctx.enter_context(tc.tile_pool(name="dram", bufs=1, space="DRAM"))
        ghpool = ctx.enter_context(tc.tile_pool(name="ghpool", bufs=G + 1))
        glpool = ctx.enter_context(tc.tile_pool(name="glpool", bufs=3))
        spool = ctx.enter_context(tc.tile_pool(name="spool", bufs=2))
        npool = ctx.enter_context(tc.tile_pool(name="npool", bufs=2))
        pwin = ctx.enter_context(tc.tile_pool(name="pwin", bufs=2, space="PSUM"))
        ppp = ctx.enter_context(tc.tile_pool(name="ppp", bufs=2, space="PSUM"))
        pnode = ctx.enter_context(tc.tile_pool(name="pnode", bufs=3, space="PSUM"))

        Tsl_q = [dram.tile([QROWS, ROW], bf16, name=f"tsl{q}") for q in range(4)]
        TfullA = dram.tile([NT, ROW], bf16, addr_space="Shared")
        TfullB = dram.tile([NT, ROW], bf16, addr_space="Shared")

        def Tslice_rows(r0, r1):
            q = r0 // QROWS
            assert (r1 - 1) // QROWS == q
            return Tsl_q[q][r0 - q * QROWS:r1 - q * QROWS, :]

        gilo_t = const.tile([128, W * CL * 8], i16)
        nc.sync.dma_start(out=gilo_t[:], in_=gilo_d[:, :])
        gihi_t = const.tile([128, W * CH * 8], i16)
        nc.sync.dma_start(out=gihi_t[:], in_=gihi_d[:, :])
        w1_t = const.tile([IN_F, 256], bf16)
        nc.sync.dma_start(out=w1_t[:], in_=w1_d[:, :])
        w2_t = const.tile([128, 2, 256], bf16)
        nc.sync.dma_start(out=w2_t[:], in_=w2_d[:, :, :])
        wo_t = const.tile([128, 2, N_CLASSES], bf16)
        nc.sync.dma_start(out=wo_t[:], in_=wo_d[:, :, :])
        alar_t = const.tile([128, 3, 2, 16], bf16)
        nc.sync.dma_start(out=alar_t[:], in_=alar_d[:, :, :, :])
        ident = const.tile([128, 128], bf16)
        make_identity(nc, ident[:])

        # zero-fill staging buffers once: trimmed pad slots are never written
        # by the gather, so they must hold finite data (sel weight is 0).
        for _ in range(G + 1):
            gi = ghpool.tile([128, CH, ROW], bf16, tag="gh")
            nc.vector.memset(gi[:], 0.0)
        for _ in range(3):
            gi = glpool.tile([128, CL, ROW], bf16, tag="gl")
            nc.vector.memset(gi[:], 0.0)
        for _ in range(2):
            ee = npool.tile([128, 128], bf16, tag="eesb")
            nc.vector.memset(ee[:], 0.0)

        def emit_table_rows(al_idx, h_T, h_node_src, t):
            """Assemble table row tile [h | el | er | 0] for node rows
            [t*128,(t+1)*128) and DMA it into the slice quarter."""
            row_t = npool.tile([128, ROW], bf16, tag="row")
            nc.scalar.activation(out=row_t[:, 0:256], in_=h_node_src, func=COPYF)
            el_ps = pnode.tile([16, 128], f32, tag="nps")
            for kt in range(2):
                nc.tensor.matmul(out=el_ps[:], lhsT=alar_t[:, al_idx, kt, :],
                                 rhs=h_T[:, kt, :], start=(kt == 0), stop=(kt == 1))
            ee_sb = npool.tile([128, 128], bf16, tag="eesb")
            nc.scalar.activation(out=ee_sb[0:16, :], in_=el_ps[:], func=COPYF)
            eeT_ps = pnode.tile([128, 128], bf16, tag="nps")
            nc.tensor.transpose(out=eeT_ps[:], in_=ee_sb[:], identity=ident[:])
            nc.scalar.activation(out=row_t[:, 256:272], in_=eeT_ps[:, 0:16], func=COPYF)
            nc.vector.memset(row_t[:, 272:ROW], 0.0)
            nc.sync.dma_start(out=Tslice_rows(t * 128, (t + 1) * 128), in_=row_t[:])

        def ag_quarter(q, Tdst):
            nc.gpsimd.collective_compute(
                "AllGather", mybir.AluOpType.bypass,
                replica_groups=[list(range(CORES))],
                ins=[Tsl_q[q].opt()],
                outs=[Tdst[q * QBLK:(q + 1) * QBLK, :]])

        # ---- P0: layer-1 table from x (reversed-quarter tile order) ----
        for i, t in enumerate(TORDER):
            xT_t = npool.tile([128, 128], bf16, tag="xTt")
            nc.sync.dma_start(out=xT_t[:], in_=xT_d[:, t * 128:(t + 1) * 128])
            h_ps = pnode.tile([128, 2, 128], f32, tag="nps")
            for mt in range(2):
                nc.tensor.matmul(out=h_ps[:, mt, :], lhsT=w1_t[:, mt * 128:(mt + 1) * 128],
                                 rhs=xT_t[:], start=True, stop=True)
            h_T = npool.tile([128, 2, 128], bf16, tag="hT")
            nc.scalar.activation(out=h_T[:], in_=h_ps[:], func=COPYF)
            hb_ps = pnode.tile([128, 2, 128], bf16, tag="nps")
            for t2 in range(2):
                nc.tensor.transpose(out=hb_ps[:, t2, :], in_=h_T[:, t2, :], identity=ident[:])
            hb = npool.tile([128, 256], f32, tag="hb")
            nc.scalar.activation(out=hb[:], in_=hb_ps[:].rearrange("p a b -> p (a b)"),
                                 func=COPYF)
            emit_table_rows(0, h_T, hb[:], t)
            if i in (12, 25, 38, 51):
                ag_quarter(3 - i // 13, TfullA)

        # ---- 3 layers of windowed edge aggregation ----
        for l in range(3):
            Tf = TfullA if l % 2 == 0 else TfullB
            Tnext = TfullB if l % 2 == 0 else TfullA

            gh_q = deque()

            def issue_high(w, Tf=Tf):
                gh = ghpool.tile([128, CH, ROW], bf16, tag="gh")
                nc.gpsimd.dma_gather(
                    out_ap=gh[:], in_ap=Tf[HALF:NT, :],
                    idxs_ap=gihi_t[:, w * CH * 8:(w + 1) * CH * 8],
                    num_idxs=CH * 128, num_idxs_reg=KH[w] * 128, elem_size=ROW,
                    single_packet=False)
                gh_q.append(gh)

            for j in range(G):
                issue_high(WORDER[j])

            for i, w in enumerate(WORDER):
                if i + G < W:
                    issue_high(WORDER[i + G])
                gl = glpool.tile([128, CL, ROW], bf16, tag="gl")
                nc.gpsimd.dma_gather(
                    out_ap=gl[:], in_ap=Tf[0:HALF, :],
                    idxs_ap=gilo_t[:, w * CL * 8:(w + 1) * CL * 8],
                    num_idxs=CL * 128, num_idxs_reg=KL[w] * 128, elem_size=ROW,
                    single_packet=False)
                gh = gh_q.popleft()

                selw = spool.tile([128, CPW, 128], bf16, tag="selw")
                nc.sync.dma_start(out=selw[:], in_=selS_d[:, w * CPW * 128:(w + 1) * CPW * 128])
                selTw = spool.tile([128, CPW, 128], bf16, tag="selTw")
                nc.sync.dma_start(out=selTw[:], in_=selT_d[:, w * CPW * 128:(w + 1) * CPW * 128])
                err_t = spool.tile([128, 8], bf16, tag="err")
                nc.sync.dma_start(out=err_t[:], in_=Tslice_rows(w * 128, (w + 1) * 128)[:, 264:272])

                # er of each slot's dst, expanded edge-wise via one-hot matmul
                pp_ps = ppp.tile([128, CPW, 8], f32, tag="pp")
                for c in range(CPW):
                    nc.tensor.matmul(out=pp_ps[:, c, :], lhsT=selw[:, c, :],
                                     rhs=err_t[:], start=True, stop=True)
                # e = el_src + er_dst; ex = exp(lrelu(e)) = max(exp(e), exp(0.2e))
                ef = spool.tile([128, CPW, 8], f32, tag="ef")
                nc.vector.tensor_tensor(out=ef[:, 0:CL, :], in0=gl[:, :, 256:264],
                                        in1=pp_ps[:, 0:CL, :], op=mybir.AluOpType.add)
                nc.vector.tensor_tensor(out=ef[:, CL:CPW, :], in0=gh[:, :, 256:264],
                                        in1=pp_ps[:, CL:CPW, :], op=mybir.AluOpType.add)
                ex1 = spool.tile([128, CPW, 8], bf16, tag="ex1")
                nc.scalar.activation(out=ex1[:], in_=ef[:], func=EXPF)
                ex2 = spool.tile([128, CPW, 8], bf16, tag="ex2")
                nc.scalar.activation(out=ex2[:], in_=ef[:], func=EXPF, scale=NEG_SLOPE)
                rhs_w = spool.tile([128, CPW, 264], bf16, tag="rhsw")
                nc.vector.tensor_tensor(out=rhs_w[:, :, 256:264], in0=ex1[:], in1=ex2[:],
                                        op=mybir.AluOpType.max)
                if l < 2:
                    for gt, c0, c1 in ((gl, 0, CL), (gh, CL, CPW)):
                        nc.vector.tensor_tensor(
                            out=rhs_w[:, c0:c1, 0:256].rearrange("p c (f h) -> p c f h", h=8),
                            in0=gt[:, :, 0:256].rearrange("p c (f h) -> p c f h", h=8),
                            in1=rhs_w[:, c0:c1, 256:264].rearrange("p c (o h) -> p c o h", o=1)
                                .to_broadcast([128, c1 - c0, 32, 8]),
                            op=mybir.AluOpType.mult)
                else:
                    for gt, c0, c1 in ((gl, 0, CL), (gh, CL, CPW)):
                        nc.vector.tensor_tensor(
                            out=rhs_w[:, c0:c1, 0:256],
                            in0=gt[:, :, 0:256],
                            in1=rhs_w[:, c0:c1, 256:257].to_broadcast([128, c1 - c0, 256]),
                            op=mybir.AluOpType.mult)
                agg_ps = pwin.tile([128, 264], f32, tag="agg")
                for c in range(CPW):
                    nc.tensor.matmul(out=agg_ps[:], lhsT=selTw[:, c, :], rhs=rhs_w[:, c, :],
                                     start=(c == 0), stop=(c == CPW - 1))

                # ---- per-window node phase ----
                if l < 2:
                    s_rec = npool.tile([128, 8], f32, tag="srec")
                    nc.vector.tensor_scalar_add(s_rec[:], agg_ps[:, 256:264], 1e-16)
                    nc.vector.reciprocal(out=s_rec[:], in_=s_rec[:])
                    u_t = npool.tile([128, 256], f32, tag="ut")
                    nc.vector.tensor_tensor(
                        out=u_t[:].rearrange("p (f h) -> p f h", h=8),
                        in0=agg_ps[:, 0:256].rearrange("p (f h) -> p f h", h=8),
                        in1=s_rec[:].rearrange("p (o h) -> p o h", o=1).to_broadcast([128, 32, 8]),
                        op=mybir.AluOpType.mult)
                    # elu(x) = exp(min(x,0)) - 1 + relu(x)
                    m0 = npool.tile([128, 256], f32, tag="m0")
                    nc.vector.tensor_scalar_min(m0[:], u_t[:], 0.0)
                    e0 = npool.tile([128, 256], f32, tag="e0")
                    nc.scalar.activation(out=e0[:], in_=m0[:], func=EXPF)
                    r0 = npool.tile([128, 256], f32, tag="r0")
                    nc.vector.tensor_scalar_max(r0[:], u_t[:], 0.0)
                    nc.vector.tensor_tensor(out=e0[:], in0=e0[:], in1=r0[:],
                                            op=mybir.AluOpType.add)
                    nc.vector.tensor_scalar_add(e0[:], e0[:], -1.0)
                    ub = npool.tile([128, 256], bf16, tag="ub")
                    nc.scalar.activation(out=ub[:], in_=e0[:], func=COPYF)
                    uT_ps = pnode.tile([128, 2, 128], bf16, tag="nps")
                    for t2 in range(2):
                        nc.tensor.transpose(out=uT_ps[:, t2, :], in_=ub[:, t2 * 128:(t2 + 1) * 128],
                                            identity=ident[:])
                    uT = npool.tile([128, 2, 128], bf16, tag="uT")
                    nc.scalar.activation(out=uT[:], in_=uT_ps[:], func=COPYF)
                    if l == 0:
                        h_ps = pnode.tile([128, 2, 128], f32, tag="nps")
                        for mt in range(2):
                            for kt in range(2):
                                nc.tensor.matmul(out=h_ps[:, mt, :],
                                                 lhsT=w2_t[:, kt, mt * 128:(mt + 1) * 128],
                                                 rhs=uT[:, kt, :],
                                                 start=(kt == 0), stop=(kt == 1))
                        h_T = npool.tile([128, 2, 128], bf16, tag="hT")
                        nc.scalar.activation(out=h_T[:], in_=h_ps[:], func=COPYF)
                        hb_ps = pnode.tile([128, 2, 128], bf16, tag="nps")
                        for t2 in range(2):
                            nc.tensor.transpose(out=hb_ps[:, t2, :], in_=h_T[:, t2, :],
                                                identity=ident[:])
                        hb = npool.tile([128, 256], f32, tag="hb")
                        nc.scalar.activation(out=hb[:], in_=hb_ps[:].rearrange("p a b -> p (a b)"),
                                             func=COPYF)
                        emit_table_rows(1, h_T, hb[:], w)
                    else:
                        emit_table_rows(2, uT, e0[:], w)
                else:
                    s_rec = npool.tile([128, 1], f32, tag="srec3")
                    nc.vector.tensor_scalar_add(s_rec[:], agg_ps[:, 256:257], 1e-16)
                    nc.vector.reciprocal(out=s_rec[:], in_=s_rec[:])
                    u_t = npool.tile([128, 256], f32, tag="ut")
                    nc.vector.tensor_tensor(
                        out=u_t[:], in0=agg_ps[:, 0:256],
                        in1=s_rec[:].to_broadcast([128, 256]), op=mybir.AluOpType.mult)
                    ub = npool.tile([128, 256], bf16, tag="ub")
                    nc.scalar.activation(out=ub[:], in_=u_t[:], func=COPYF)
                    uT_ps = pnode.tile([128, 2, 128], bf16, tag="nps")
                    for t2 in range(2):
                        nc.tensor.transpose(out=uT_ps[:, t2, :], in_=ub[:, t2 * 128:(t2 + 1) * 128],
                                            identity=ident[:])
                    uT = npool.tile([128, 2, 128], bf16, tag="uT")
                    nc.scalar.activation(out=uT[:], in_=uT_ps[:], func=COPYF)
                    o_ps = pnode.tile([N_CLASSES, 128], f32, tag="nps")
                    for kt in range(2):
                        nc.tensor.matmul(out=o_ps[:], lhsT=wo_t[:, kt, :],
                                         rhs=uT[:, kt, :], start=(kt == 0), stop=(kt == 1))
                    ob = npool.tile([128, 128], bf16, tag="ob")
                    nc.vector.memset(ob[:], 0.0)
                    nc.vector.tensor_copy(out=ob[0:N_CLASSES, :], in_=o_ps[:])
                    on_ps = pnode.tile([128, 128], bf16, tag="nps")
                    nc.tensor.transpose(out=on_ps[:], in_=ob[:], identity=ident[:])
                    o_n = npool.tile([128, N_CLASSES], f32, tag="on")
                    nc.vector.tensor_copy(out=o_n[:], in_=on_ps[:, 0:N_CLASSES])
                    mx = npool.tile([128, 1], f32, tag="mx")
                    nc.vector.tensor_reduce(out=mx[:], in_=o_n[:], axis=mybir.AxisListType.X,
                                            op=mybir.AluOpType.max)
                    nc.vector.tensor_tensor(out=o_n[:], in0=o_n[:],
                                            in1=mx[:].to_broadcast([128, N_CLASSES]),
                                            op=mybir.AluOpType.subtract)
                    ex_t = npool.tile([128, N_CLASSES], f32, tag="ext")
                    nc.scalar.activation(out=ex_t[:], in_=o_n[:], func=EXPF)
                    sm = npool.tile([128, 1], f32, tag="sm")
                    nc.vector.tensor_reduce(out=sm[:], in_=ex_t[:], axis=mybir.AxisListType.X,
                                            op=mybir.AluOpType.add)
                    ln_t = npool.tile([128, 1], f32, tag="lnt")
                    nc.scalar.activation(out=ln_t[:], in_=sm[:], func=LNF)
                    res = npool.tile([128, N_CLASSES], f32, tag="res")
                    nc.vector.tensor_tensor(out=res[:], in0=o_n[:],
                                            in1=ln_t[:].to_broadcast([128, N_CLASSES]),
                                            op=mybir.AluOpType.subtract)
                    nc.sync.dma_start(out=outy[w * 128:(w + 1) * 128, :], in_=res[:])

                # partial AllGathers overlapped with later windows
                if l < 2 and i in (11, 24, 37):
                    ag_quarter({11: 3, 24: 2, 37: 1}[i], Tnext)
            if l < 2:
                ag_quarter(0, Tnext)

    nc.compile()
    return nc


_CACHE = {}
_LAST = {}


def _make_in_maps(ins, per_core):
    x = np.asarray(ins["x"], np.float32)
    W1n = np.asarray(ins["W1"], np.float32)
    W2n = np.asarray(ins["W2"], np.float32)
    Won = np.asarray(ins["Wo"], np.float32)
    al1n, ar1n = np.asarray(ins["al1"], np.float32), np.asarray(ins["ar1"], np.float32)
    al2n, ar2n = np.asarray(ins["al2"], np.float32), np.asarray(ins["ar2"], np.float32)
    alon, aron = np.asarray(ins["alo"], np.float32), np.asarray(ins["aro"], np.float32)

    W1p = W1n[:, PERM]
    W2p = W2n[PERM][:, PERM]
    Wop = Won[PERM]

    w1_h = W1p.astype(BF16)                                          # [128, 256]
    w2_h = np.ascontiguousarray(
        W2p.reshape(2, 128, 256).transpose(1, 0, 2)).astype(BF16)    # [128, 2, 256]
    wo_h = np.ascontiguousarray(
        Wop.reshape(2, 128, N_CLASSES).transpose(1, 0, 2)).astype(BF16)
    alar = np.zeros((3, 256, 16), np.float32)
    alar[0] = _alar_block(al1n, ar1n, 256)
    alar[1] = _alar_block(al2n, ar2n, 256)
    alar[2][:, 0] = Won @ alon[0]
    alar[2][:, 8] = Won @ aron[0]
    alar = alar[:, PERM, :]
    alar_h = np.ascontiguousarray(
        alar.reshape(3, 2, 128, 16).transpose(2, 0, 1, 3)).astype(BF16)  # [128, 3, 2, 16]

    in_maps = []
    for c in range(CORES):
        xs = np.zeros((SP, IN_F), np.float32)
        xs[:SPR] = x[c * SPR:(c + 1) * SPR]
        xT_h = np.ascontiguousarray(xs.T).astype(BF16)               # [128, SP]
        pc = per_core[c]
        in_maps.append(dict(
            xT=xT_h, gilo=pc["gilo"], gihi=pc["gihi"],
            selS=pc["selS"], selT=pc["selT"],
            w1=w1_h, w2=w2_h, wo=wo_h, alar=alar_h))
    return in_maps


def kernel(x, src, dst, W1, al1, ar1, W2, al2, ar2, Wo, alo, aro):
    src = np.asarray(src, np.int32)
    dst = np.asarray(dst, np.int32)

    per_core, CL, CH, CPW, KL, KH = _host_prep(src, dst)

    key = (CL, CH, tuple(KL), tuple(KH))
    if key not in _CACHE:
        _CACHE[key] = _build_program(CL, CH, CPW, KL, KH)
    nc = _CACHE[key]

    in_maps = _make_in_maps(
        dict(x=x, W1=W1, al1=al1, ar1=ar1, W2=W2, al2=al2, ar2=ar2,
             Wo=Wo, alo=alo, aro=aro), per_core)

    _LAST["nc"] = nc
    _LAST["in_maps"] = in_maps
    res = run_bass_kernel_spmd(nc, in_maps, list(range(CORES)))
    out = np.concatenate([res.results[c]["outy"][:SPR] for c in range(CORES)], axis=0)
    return out.astype(np.float32)


# revision 22
# speedup vs baseline: 39410.9141x; 1.1798x over previous
"""3-layer GAT on Trainium2, 8 NeuronCores — v2.

Strategy (dst-sharded, replicated tables). The hard resource is the GpSimd
Q7 cluster: every dma_gather descriptor costs ~8-10ns of Q7 descriptor
generation, and the kernel needs one gathered table row per edge per layer
(~100k rows/core/layer). v2 therefore (a) trims pad slots via the Q7
kernel's trailing-negative-index trim, (b) keeps Q7 busy through layer
boundaries: tables AllGather in QUARTERS, windows are processed in
reversed-quarter order so early quarters of the next table finish mid-layer,
and each window's high-half gather (table rows >= 26624) runs G windows
ahead of its low-half gather, bridging the table-ready gap.

Table row (768B, bf16): [h(256, (f,h)-interleaved) | el(8) | er(8) | pad96].
Edge weights: ex = exp(leaky_relu(el_src + er_dst)) computed exactly as
max(exp(e), exp(0.2 e)). Softmax normalization happens after aggregation
(alpha = ex/sum(ex) is scale-invariant per dst; |logits| < 10 so no
segment-max needed). Layer 3 commutes the output projection with
aggregation: sum(ex*h2) @ Wo == sum(ex*(h2@Wo)).
"""
import numpy as np
import ml_dtypes
from collections import deque
from contextlib import ExitStack

import concourse.bass as bass
import concourse.tile as tile
from concourse import bacc, mybir
from concourse.bass_utils import run_bass_kernel_spmd
from concourse.masks import make_identity

BF16 = ml_dtypes.bfloat16

N_NODES = 50000
IN_F = 128
N_CLASSES = 40
CORES = 8
SPR = 6250          # real dst nodes per core
SP = 6656           # slice rows per core (52 * 128)
NT = SP * CORES     # 53248 padded table rows
W = 49              # dst windows per core (ceil(6250/128))
ROW = 384           # table row elems (bf16 -> 768B, multiple of 256B)
NTILE = SP // 128   # 52 node tiles per slice
NEG_SLOPE = 0.2
EXPF = mybir.ActivationFunctionType.Exp
COPYF = mybir.ActivationFunctionType.Copy
LNF = mybir.ActivationFunctionType.Ln

QT = NTILE // 4      # 13 tiles per table quarter
QROWS = QT * 128     # 1664 slice rows per quarter
QBLK = QROWS * CORES # 13312 table rows per quarter
HALF = 2 * QBLK      # 26624: gather low/high split == quarters {0,1}/{2,3}
G = 5                # high-gather lookahead (windows)

# reversed-quarter processing order: quarter q3's windows first, so the next
# table's quarters complete (and AllGather) as early as possible.
WORDER = list(range(39, 49)) + list(range(26, 39)) + list(range(13, 26)) + list(range(0, 13))
TORDER = list(range(39, 52)) + list(range(26, 39)) + list(range(13, 26)) + list(range(0, 13))

# h column interleave: new col k holds old col (k%8)*32 + (k//8), i.e.
# feature-major with the 8 heads contiguous. Lets per-head edge weights
# broadcast with a unit-stride inner dim of 8 on the DVE.
PERM = np.array([(k % 8) * 32 + (k // 8) for k in range(256)], np.int64)


def _remap(n):
    """Global table row for node n, laid out (quarter, core, tile, row) so
    quarter-wise partial AllGathers are contiguous in slice and table."""
    c = n // SPR
    r = n % SPR
    t = r // 128
    q = t // QT
    return q * QBLK + c * QROWS + (t % QT) * 128 + (r % 128)


def _wrap16(vals, nidx):
    """dma_gather index layout: flat idx i -> [i%16, i//16], replicated to
    all 8 groups of 16 partitions."""
    blk = np.zeros((16, nidx // 16), np.int16)
    blk[np.arange(nidx) % 16, np.arange(nidx) // 16] = vals
    return np.tile(blk, (8, 1))


def _host_prep(src, dst):
    gsrc = _remap(src.astype(np.int64))
    d64 = dst.astype(np.int64)
    core = d64 // SPR
    ld = d64 % SPR
    w = ld >> 7
    dstl = (ld & 127).astype(np.int64)
    ishigh = (gsrc >= HALF).astype(np.int64)
    wg = core * W + w

    order = np.lexsort((ishigh, wg))
    gsrc_s, dstl_s, wg_s, hi_s = gsrc[order], dstl[order], wg[order], ishigh[order]

    gkey = wg_s * 2 + hi_s
    uniq, starts, counts = np.unique(gkey, return_index=True, return_counts=True)
    pos_in_grp = np.arange(len(gkey)) - np.repeat(starts, counts)

    nlow = np.zeros(CORES * W, np.int64)
    nhigh = np.zeros(CORES * W, np.int64)
    for u, c in zip(uniq, counts):
        (nlow if u % 2 == 0 else nhigh)[u // 2] = c
    # Variable per-window gather regions (compile-time constants, shared
    # program): KL[w] = low chunks needed by the worst core for window w.
    KL = [max(1, int(np.ceil(max(nlow[c * W + w2] for c in range(CORES)) / 128)))
          for w2 in range(W)]
    KH = [max(1, int(np.ceil(max(nhigh[c * W + w2] for c in range(CORES)) / 128)))
          for w2 in range(W)]
    KW = [KL[w2] + KH[w2] for w2 in range(W)]
    # packed offsets, in chunks
    OFFL = np.concatenate([[0], np.cumsum(KL)]).astype(np.int64)
    OFFH = np.concatenate([[0], np.cumsum(KH)]).astype(np.int64)
    OFFW = np.concatenate([[0], np.cumsum(KW)]).astype(np.int64)

    w_local = wg_s % W
    # slot relative to the window's packed region
    rel = np.where(hi_s == 1, np.asarray(KL)[w_local] * 128, 0) + pos_in_grp
    slot = OFFW[w_local] * 128 + rel
    core_s = wg_s // W

    NSLOT = int(OFFW[-1]) * 128
    NLOW = int(OFFL[-1]) * 128
    NHIGH = int(OFFH[-1]) * 128
    per_core = []
    for c in range(CORES):
        m = core_s == c
        gs = gsrc_s[m]
        dl = dstl_s[m]
        hi = hi_s[m]
        wl = w_local[m]
        rl = rel[m]
        sl = slot[m]

        lowidx = np.zeros(NLOW, np.int16)
        highidx = np.zeros(NHIGH, np.int16)
        lm = hi == 0
        lowidx[OFFL[wl[lm]] * 128 + rl[lm]] = gs[lm].astype(np.int16)
        hm = hi == 1
        highidx[OFFH[wl[hm]] * 128 + (rl[hm] - np.asarray(KL)[wl[hm]] * 128)] = \
            (gs[hm] - HALF).astype(np.int16)

        gilo = np.concatenate(
            [_wrap16(lowidx[OFFL[i] * 128:OFFL[i + 1] * 128], KL[i] * 128)
             for i in range(W)], axis=1)
        gihi = np.concatenate(
            [_wrap16(highidx[OFFH[i] * 128:OFFH[i + 1] * 128], KH[i] * 128)
             for i in range(W)], axis=1)

        # one-hot streams; pad slots keep all-zero column/row -> zero contribution
        selS = np.zeros((128, NSLOT), BF16)    # sel[d, slot]
        selT = np.zeros((128, NSLOT), BF16)    # selT[e_lane, chunk*128 + d]
        e_lane = sl % 128
        chunk = sl // 128
        selS[dl, sl] = 1
        selT[e_lane, chunk * 128 + dl] = 1

        per_core.append(dict(gilo=gilo, gihi=gihi, selS=selS, selT=selT))
    return per_core, KL, KH


def _alar_block(al, ar, fout):
    """[fout, 16]: col j (<8) extracts el head j, col j+8 er head j."""
    H, F = al.shape
    m = np.zeros((fout, 16), np.float32)
    for j in range(H):
        m[j * F:(j + 1) * F, j] = al[j]
        m[j * F:(j + 1) * F, j + 8] = ar[j]
    return m


def _build_program(KL, KH):
    KW = [KL[w] + KH[w] for w in range(W)]
    OFFL = [0]
    OFFH = [0]
    OFFW = [0]
    for w in range(W):
        OFFL.append(OFFL[-1] + KL[w])
        OFFH.append(OFFH[-1] + KH[w])
        OFFW.append(OFFW[-1] + KW[w])
    CLmax, CHmax, CPWmax = max(KL), max(KH), max(KW)

    nc = bacc.Bacc("TRN2", target_bir_lowering=False, debug=False, num_devices=CORES)
    f32, bf16, i16 = mybir.dt.float32, mybir.dt.bfloat16, mybir.dt.int16

    xT_d = nc.declare_dram_parameter("xT", [IN_F, SP], bf16, isOutput=False)
    gilo_d = nc.declare_dram_parameter("gilo", [128, OFFL[-1] * 8], i16, isOutput=False)
    gihi_d = nc.declare_dram_parameter("gihi", [128, OFFH[-1] * 8], i16, isOutput=False)
    selS_d = nc.declare_dram_parameter("selS", [128, OFFW[-1] * 128], bf16, isOutput=False)
    selT_d = nc.declare_dram_parameter("selT", [128, OFFW[-1] * 128], bf16, isOutput=False)
    w1_d = nc.declare_dram_parameter("w1", [IN_F, 256], bf16, isOutput=False)
    w2_d = nc.declare_dram_parameter("w2", [128, 2, 256], bf16, isOutput=False)
    wo_d = nc.declare_dram_parameter("wo", [128, 2, N_CLASSES], bf16, isOutput=False)
    alar_d = nc.declare_dram_parameter("alar", [128, 3, 2, 16], bf16, isOutput=False)
    outy = nc.declare_dram_parameter("outy", [SP, N_CLASSES], f32, isOutput=True)

    with ExitStack() as ctx:
        tc = ctx.enter_context(tile.TileContext(nc))
        const = ctx.enter_context(tc.tile_pool(name="const", bufs=1))
        dram = ctx.enter_context(tc.tile_pool(name="dram", bufs=1, space="DRAM"))
        ghpool = ctx.enter_context(tc.tile_pool(name="ghpool", bufs=G + 1))
        glpool = ctx.enter_context(tc.tile_pool(name="glpool", bufs=3))
        spool = ctx.enter_context(tc.tile_pool(name="spool", bufs=2))
        npool = ctx.enter_context(tc.tile_pool(name="npool", bufs=2))
        pwin = ctx.enter_context(tc.tile_pool(name="pwin", bufs=2, space="PSUM"))
        ppp = ctx.enter_context(tc.tile_pool(name="ppp", bufs=2, space="PSUM"))
        pnode = ctx.enter_context(tc.tile_pool(name="pnode", bufs=3, space="PSUM"))

        Tsl_q = [dram.tile([QROWS, ROW], bf16, name=f"tsl{q}") for q in range(4)]
        TfullA = dram.tile([NT, ROW], bf16)
        TfullB = dram.tile([NT, ROW], bf16)

        def Tslice_rows(r0, r1):
            q = r0 // QROWS
            assert (r1 - 1) // QROWS == q
            return Tsl_q[q][r0 - q * QROWS:r1 - q * QROWS, :]

        gilo_t = const.tile([128, OFFL[-1] * 8], i16)
        nc.sync.dma_start(out=gilo_t[:], in_=gilo_d[:, :])
        gihi_t = const.tile([128, OFFH[-1] * 8], i16)
        nc.sync.dma_start(out=gihi_t[:], in_=gihi_d[:, :])
        w1_t = const.tile([IN_F, 256], bf16)
        nc.sync.dma_start(out=w1_t[:], in_=w1_d[:, :])
        w2_t = const.tile([128, 2, 256], bf16)
        nc.sync.dma_start(out=w2_t[:], in_=w2_d[:, :, :])
        wo_t = const.tile([128, 2, N_CLASSES], bf16)
        nc.sync.dma_start(out=wo_t[:], in_=wo_d[:, :, :])
        alar_t = const.tile([128, 3, 2, 16], bf16)
        nc.sync.dma_start(out=alar_t[:], in_=alar_d[:, :, :, :])
        ident = const.tile([128, 128], bf16)
        make_identity(nc, ident[:])

        # zero-fill staging buffers once: chunks beyond a window's region are
        # never written by its gather, so they must hold finite data.
        for _ in range(G + 1):
            gi = ghpool.tile([128, CHmax, ROW], bf16, tag="gh")
            nc.vector.memset(gi[:], 0.0)
        for _ in range(3):
            gi = glpool.tile([128, CLmax, ROW], bf16, tag="gl")
            nc.vector.memset(gi[:], 0.0)
        for _ in range(2):
            ee = npool.tile([128, 128], bf16, tag="eesb")
            nc.vector.memset(ee[:], 0.0)

        def emit_table_rows(al_idx, h_T, h_node_src, t):
            """Assemble table row tile [h | el | er | 0] for node rows
            [t*128,(t+1)*128) and DMA it into the slice quarter."""
            row_t = npool.tile([128, ROW], bf16, tag="row")
            nc.scalar.activation(out=row_t[:, 0:256], in_=h_node_src, func=COPYF)
            el_ps = pnode.tile([16, 128], f32, tag="nps")
            for kt in range(2):
                nc.tensor.matmul(out=el_ps[:], lhsT=alar_t[:, al_idx, kt, :],
                                 rhs=h_T[:, kt, :], start=(kt == 0), stop=(kt == 1))
            ee_sb = npool.tile([128, 128], bf16, tag="eesb")
            nc.scalar.activation(out=ee_sb[0:16, :], in_=el_ps[:], func=COPYF)
            eeT_ps = pnode.tile([128, 128], bf16, tag="nps")
            nc.tensor.transpose(out=eeT_ps[:], in_=ee_sb[:], identity=ident[:])
            nc.scalar.activation(out=row_t[:, 256:272], in_=eeT_ps[:, 0:16], func=COPYF)
            nc.sync.dma_start(out=Tslice_rows(t * 128, (t + 1) * 128), in_=row_t[:])

        def ag_quarter(q, Tdst):
            nc.gpsimd.collective_compute(
                "AllGather", mybir.AluOpType.bypass,
                replica_groups=[list(range(CORES))],
                ins=[Tsl_q[q].opt()],
                outs=[Tdst[q * QBLK:(q + 1) * QBLK, :]])

        # ---- P0: layer-1 table from x (reversed-quarter tile order) ----
        for i, t in enumerate(TORDER):
            xT_t = npool.tile([128, 128], bf16, tag="xTt")
            nc.sync.dma_start(out=xT_t[:], in_=xT_d[:, t * 128:(t + 1) * 128])
            h_ps = pnode.tile([128, 2, 128], f32, tag="nps")
            for mt in range(2):
                nc.tensor.matmul(out=h_ps[:, mt, :], lhsT=w1_t[:, mt * 128:(mt + 1) * 128],
                                 rhs=xT_t[:], start=True, stop=True)
            h_T = npool.tile([128, 2, 128], bf16, tag="hT")
            nc.scalar.activation(out=h_T[:], in_=h_ps[:], func=COPYF)
            hb_ps = pnode.tile([128, 2, 128], bf16, tag="nps")
            for t2 in range(2):
                nc.tensor.transpose(out=hb_ps[:, t2, :], in_=h_T[:, t2, :], identity=ident[:])
            hb = npool.tile([128, 256], f32, tag="hb")
            nc.scalar.activation(out=hb[:], in_=hb_ps[:].rearrange("p a b -> p (a b)"),
                                 func=COPYF)
            emit_table_rows(0, h_T, hb[:], t)
            if i in (12, 25, 38, 51):
                ag_quarter(3 - i // 13, TfullA)

        # ---- 3 layers of windowed edge aggregation ----
        for l in range(3):
            Tf = TfullA if l % 2 == 0 else TfullB
            Tnext = TfullB if l % 2 == 0 else TfullA

            gh_q = deque()

            def issue_high(w, Tf=Tf):
                gh = ghpool.tile([128, CHmax, ROW], bf16, tag="gh")
                nc.gpsimd.dma_gather(
                    out_ap=gh[:, 0:KH[w], :], in_ap=Tf[HALF:NT, :],
                    idxs_ap=gihi_t[:, OFFH[w] * 8:OFFH[w + 1] * 8],
                    num_idxs=KH[w] * 128, num_idxs_reg=KH[w] * 128, elem_size=ROW,
                    single_packet=False)
                gh_q.append(gh)

            for j in range(G):
                issue_high(WORDER[j])

            for i, w in enumerate(WORDER):
                if i + G < W:
                    issue_high(WORDER[i + G])
                kl, kh, kw = KL[w], KH[w], KW[w]
                gl = glpool.tile([128, CLmax, ROW], bf16, tag="gl")
                nc.gpsimd.dma_gather(
                    out_ap=gl[:, 0:kl, :], in_ap=Tf[0:HALF, :],
                    idxs_ap=gilo_t[:, OFFL[w] * 8:OFFL[w + 1] * 8],
                    num_idxs=kl * 128, num_idxs_reg=kl * 128, elem_size=ROW,
                    single_packet=False)
                gh = gh_q.popleft()

                selw = spool.tile([128, CPWmax, 128], bf16, tag="selw")
                nc.sync.dma_start(out=selw[:, 0:kw, :],
                                  in_=selS_d[:, OFFW[w] * 128:OFFW[w + 1] * 128])
                selTw = spool.tile([128, CPWmax, 128], bf16, tag="selTw")
                nc.sync.dma_start(out=selTw[:, 0:kw, :],
                                  in_=selT_d[:, OFFW[w] * 128:OFFW[w + 1] * 128])
                err_t = spool.tile([128, 8], bf16, tag="err")
                nc.sync.dma_start(out=err_t[:], in_=Tslice_rows(w * 128, (w + 1) * 128)[:, 264:272])

                # er of each slot's dst, expanded edge-wise via one-hot matmul
                pp_ps = ppp.tile([128, CPWmax, 8], f32, tag="pp")
                for c in range(kw):
                    nc.tensor.matmul(out=pp_ps[:, c, :], lhsT=selw[:, c, :],
                                     rhs=err_t[:], start=True, stop=True)
                # e = el_src + er_dst; ex = exp(lrelu(e)) = max(exp(e), exp(0.2e))
                ef = spool.tile([128, CPWmax, 8], f32, tag="ef")
                nc.vector.tensor_tensor(out=ef[:, 0:kl, :], in0=gl[:, 0:kl, 256:264],
                                        in1=pp_ps[:, 0:kl, :], op=mybir.AluOpType.add)
                nc.vector.tensor_tensor(out=ef[:, kl:kw, :], in0=gh[:, 0:kh, 256:264],
                                        in1=pp_ps[:, kl:kw, :], op=mybir.AluOpType.add)
                ex1 = spool.tile([128, CPWmax, 8], bf16, tag="ex1")
                nc.scalar.activation(out=ex1[:, 0:kw, :], in_=ef[:, 0:kw, :], func=EXPF)
                ex2 = spool.tile([128, CPWmax, 8], bf16, tag="ex2")
                nc.scalar.activation(out=ex2[:, 0:kw, :], in_=ef[:, 0:kw, :], func=EXPF,
                                     scale=NEG_SLOPE)
                rhs_w = spool.tile([128, CPWmax, 264], bf16, tag="rhsw")
                nc.vector.tensor_tensor(out=rhs_w[:, 0:kw, 256:264], in0=ex1[:, 0:kw, :],
                                        in1=ex2[:, 0:kw, :], op=mybir.AluOpType.max)
                if l < 2:
                    for gt, c0, c1 in ((gl, 0, kl), (gh, kl, kw)):
                        nc.vector.tensor_tensor(
                            out=rhs_w[:, c0:c1, 0:256].rearrange("p c (f h) -> p c f h", h=8),
                            in0=gt[:, 0:c1 - c0, 0:256].rearrange("p c (f h) -> p c f h", h=8),
                            in1=rhs_w[:, c0:c1, 256:264].rearrange("p c (o h) -> p c o h", o=1)
                                .to_broadcast([128, c1 - c0, 32, 8]),
                            op=mybir.AluOpType.mult)
                else:
                    for gt, c0, c1 in ((gl, 0, kl), (gh, kl, kw)):
                        nc.vector.tensor_tensor(
                            out=rhs_w[:, c0:c1, 0:256],
                            in0=gt[:, 0:c1 - c0, 0:256],
                            in1=rhs_w[:, c0:c1, 256:257].to_broadcast([128, c1 - c0, 256]),
                            op=mybir.AluOpType.mult)
                agg_ps = pwin.tile([128, 264], f32, tag="agg")
                for c in range(kw):
                    nc.tensor.matmul(out=agg_ps[:], lhsT=selTw[:, c, :], rhs=rhs_w[:, c, :],
                                     start=(c == 0), stop=(c == kw - 1))

                # ---- per-window node phase ----
                if l < 2:
                    s_rec = npool.tile([128, 8], f32, tag="srec")
                    nc.vector.tensor_scalar_add(s_rec[:], agg_ps[:, 256:264], 1e-16)
                    nc.vector.reciprocal(out=s_rec[:], in_=s_rec[:])
                    u_t = npool.tile([128, 256], f32, tag="ut")
                    nc.vector.tensor_tensor(
                        out=u_t[:].rearrange("p (f h) -> p f h", h=8),
                        in0=agg_ps[:, 0:256].rearrange("p (f h) -> p f h", h=8),
                        in1=s_rec[:].rearrange("p (o h) -> p o h", o=1).to_broadcast([128, 32, 8]),
                        op=mybir.AluOpType.mult)
                    # elu(x) = exp(min(x,0)) - 1 + relu(x)
                    m0 = npool.tile([128, 256], f32, tag="m0")
                    nc.vector.tensor_scalar_min(m0[:], u_t[:], 0.0)
                    e0 = npool.tile([128, 256], f32, tag="e0")
                    nc.scalar.activation(out=e0[:], in_=m0[:], func=EXPF)
                    r0 = npool.tile([128, 256], f32, tag="r0")
                    nc.vector.tensor_scalar_max(r0[:], u_t[:], 0.0)
                    nc.vector.tensor_tensor(out=e0[:], in0=e0[:], in1=r0[:],
                                            op=mybir.AluOpType.add)
                    nc.vector.tensor_scalar_add(e0[:], e0[:], -1.0)
                    ub = npool.tile([128, 256], bf16, tag="ub")
                    nc.scalar.activation(out=ub[:], in_=e0[:], func=COPYF)
                    uT_ps = pnode.tile([128, 2, 128], bf16, tag="nps")
                    for t2 in range(2):
                        nc.tensor.transpose(out=uT_ps[:, t2, :], in_=ub[:, t2 * 128:(t2 + 1) * 128],
                                            identity=ident[:])
                    uT = npool.tile([128, 2, 128], bf16, tag="uT")
                    nc.scalar.activation(out=uT[:], in_=uT_ps[:], func=COPYF)
                    if l == 0:
                        h_ps = pnode.tile([128, 2, 128], f32, tag="nps")
                        for mt in range(2):
                            for kt in range(2):
                                nc.tensor.matmul(out=h_ps[:, mt, :],
                                                 lhsT=w2_t[:, kt, mt * 128:(mt + 1) * 128],
                                                 rhs=uT[:, kt, :],
                                                 start=(kt == 0), stop=(kt == 1))
                        h_T = npool.tile([128, 2, 128], bf16, tag="hT")
                        nc.scalar.activation(out=h_T[:], in_=h_ps[:], func=COPYF)
                        hb_ps = pnode.tile([128, 2, 128], bf16, tag="nps")
                        for t2 in range(2):
                            nc.tensor.transpose(out=hb_ps[:, t2, :], in_=h_T[:, t2, :],
                                                identity=ident[:])
                        hb = npool.tile([128, 256], f32, tag="hb")
                        nc.scalar.activation(out=hb[:], in_=hb_ps[:].rearrange("p a b -> p (a b)"),
                                             func=COPYF)
                        emit_table_rows(1, h_T, hb[:], w)
                    else:
                        emit_table_rows(2, uT, e0[:], w)
                else:
                    s_rec = npool.tile([128, 1], f32, tag="srec3")
                    nc.vector.tensor_scalar_add(s_rec[:], agg_ps[:, 256:257], 1e-16)
                    nc.vector.reciprocal(out=s_rec[:], in_=s_rec[:])
                    u_t = npool.tile([128, 256], f32, tag="ut")
                    nc.vector.tensor_tensor(
                        out=u_t[:], in0=agg_ps[:, 0:256],
                        in1=s_rec[:].to_broadcast([128, 256]), op=mybir.AluOpType.mult)
                    ub = npool.tile([128, 256], bf16, tag="ub")
                    nc.scalar.activation(out=ub[:], in_=u_t[:], func=COPYF)
                    uT_ps = pnode.tile([128, 2, 128], bf16, tag="nps")
                    for t2 in range(2):
                        nc.tensor.transpose(out=uT_ps[:, t2, :], in_=ub[:, t2 * 128:(t2 + 1) * 128],
                                            identity=ident[:])
                    uT = npool.tile([128, 2, 128], bf16, tag="uT")
                    nc.scalar.activation(out=uT[:], in_=uT_ps[:], func=COPYF)
                    o_ps = pnode.tile([N_CLASSES, 128], f32, tag="nps")
                    for kt in range(2):
                        nc.tensor.matmul(out=o_ps[:], lhsT=wo_t[:, kt, :],
                                         rhs=uT[:, kt, :], start=(kt == 0), stop=(kt == 1))
                    ob = npool.tile([128, 128], bf16, tag="ob")
                    nc.vector.memset(ob[:], 0.0)
                    nc.vector.tensor_copy(out=ob[0:N_CLASSES, :], in_=o_ps[:])
                    on_ps = pnode.tile([128, 128], bf16, tag="nps")
                    nc.tensor.transpose(out=on_ps[:], in_=ob[:], identity=ident[:])
                    o_n = npool.tile([128, N_CLASSES], f32, tag="on")
                    nc.vector.tensor_copy(out=o_n[:], in_=on_ps[:, 0:N_CLASSES])
                    mx = npool.tile([128, 1], f32, tag="mx")
                    nc.vector.tensor_reduce(out=mx[:], in_=o_n[:], axis=mybir.AxisListType.X,
                                            op=mybir.AluOpType.max)
                    nc.vector.tensor_tensor(out=o_n[:], in0=o_n[:],
                                            in1=mx[:].to_broadcast([128, N_CLASSES]),
                                            op=mybir.AluOpType.subtract)
                    ex_t = npool.tile([128, N_CLASSES], f32, tag="ext")
                    nc.scalar.activation(out=ex_t[:], in_=o_n[:], func=EXPF)
                    sm = npool.tile([128, 1], f32, tag="sm")
                    nc.vector.tensor_reduce(out=sm[:], in_=ex_t[:], axis=mybir.AxisListType.X,
                                            op=mybir.AluOpType.add)
                    ln_t = npool.tile([128, 1], f32, tag="lnt")
                    nc.scalar.activation(out=ln_t[:], in_=sm[:], func=LNF)
                    res = npool.tile([128, N_CLASSES], f32, tag="res")
                    nc.vector.tensor_tensor(out=res[:], in0=o_n[:],
                                            in1=ln_t[:].to_broadcast([128, N_CLASSES]),
                                            op=mybir.AluOpType.subtract)
                    nc.sync.dma_start(out=outy[w * 128:(w + 1) * 128, :], in_=res[:])

                # partial AllGathers overlapped with later windows
                if l < 2 and i in (11, 24, 37):
                    ag_quarter({11: 3, 24: 2, 37: 1}[i], Tnext)
            if l < 2:
                ag_quarter(0, Tnext)

    nc.compile()
    return nc


_CACHE = {}
_LAST = {}


def _make_in_maps(ins, per_core):
    x = np.asarray(ins["x"], np.float32)
    W1n = np.asarray(ins["W1"], np.float32)
    W2n = np.asarray(ins["W2"], np.float32)
    Won = np.asarray(ins["Wo"], np.float32)
    al1n, ar1n = np.asarray(ins["al1"], np.float32), np.asarray(ins["ar1"], np.float32)
    al2n, ar2n = np.asarray(ins["al2"], np.float32), np.asarray(ins["ar2"], np.float32)
    alon, aron = np.asarray(ins["alo"], np.float32), np.asarray(ins["aro"], np.float32)

    W1p = W1n[:, PERM]
    W2p = W2n[PERM][:, PERM]
    Wop = Won[PERM]

    w1_h = W1p.astype(BF16)                                          # [128, 256]
    w2_h = np.ascontiguousarray(
        W2p.reshape(2, 128, 256).transpose(1, 0, 2)).astype(BF16)    # [128, 2, 256]
    wo_h = np.ascontiguousarray(
        Wop.reshape(2, 128, N_CLASSES).transpose(1, 0, 2)).astype(BF16)
    alar = np.zeros((3, 256, 16), np.float32)
    alar[0] = _alar_block(al1n, ar1n, 256)
    alar[1] = _alar_block(al2n, ar2n, 256)
    alar[2][:, 0] = Won @ alon[0]
    alar[2][:, 8] = Won @ aron[0]
    alar = alar[:, PERM, :]
    alar_h = np.ascontiguousarray(
        alar.reshape(3, 2, 128, 16).transpose(2, 0, 1, 3)).astype(BF16)  # [128, 3, 2, 16]

    in_maps = []
    for c in range(CORES):
        xs = np.zeros((SP, IN_F), np.float32)
        xs[:SPR] = x[c * SPR:(c + 1) * SPR]
        xT_h = np.ascontiguousarray(xs.T).astype(BF16)               # [128, SP]
        pc = per_core[c]
        in_maps.append(dict(
            xT=xT_h, gilo=pc["gilo"], gihi=pc["gihi"],
            selS=pc["selS"], selT=pc["selT"],
            w1=w1_h, w2=w2_h, wo=wo_h, alar=alar_h))
    return in_maps


def kernel(x, src, dst, W1, al1, ar1, W2, al2, ar2, Wo, alo, aro):
    src = np.asarray(src, np.int32)
    dst = np.asarray(dst, np.int32)

    per_core, KL, KH = _host_prep(src, dst)

    key = (tuple(KL), tuple(KH))
    if key not in _CACHE:
        _CACHE[key] = _build_program(KL, KH)
    nc = _CACHE[key]

    in_maps = _make_in_maps(
        dict(x=x, W1=W1, al1=al1, ar1=ar1, W2=W2, al2=al2, ar2=ar2,
             Wo=Wo, alo=alo, aro=aro), per_core)

    _LAST["nc"] = nc
    _LAST["in_maps"] = in_maps
    res = run_bass_kernel_spmd(nc, in_maps, list(range(CORES)))
    out = np.concatenate([res.results[c]["outy"][:SPR] for c in range(CORES)], axis=0)
    return out.astype(np.float32)


# revision 29
# speedup vs baseline: 42871.8983x; 1.0878x over previous
"""3-layer GAT on Trainium2, 8 NeuronCores — v2.

Strategy (dst-sharded, replicated tables). The hard resource is the GpSimd
Q7 cluster: every dma_gather descriptor costs ~8-10ns of Q7 descriptor
generation, and the kernel needs one gathered table row per edge per layer
(~100k rows/core/layer). v2 therefore (a) trims pad slots via the Q7
kernel's trailing-negative-index trim, (b) keeps Q7 busy through layer
boundaries: tables AllGather in QUARTERS, windows are processed in
reversed-quarter order so early quarters of the next table finish mid-layer,
and each window's high-half gather (table rows >= 26624) runs G windows
ahead of its low-half gather, bridging the table-ready gap.

Table row (768B, bf16): [h(256, (f,h)-interleaved) | el(8) | er(8) | pad96].
Edge weights: ex = exp(leaky_relu(el_src + er_dst)) computed exactly as
max(exp(e), exp(0.2 e)). Softmax normalization happens after aggregation
(alpha = ex/sum(ex) is scale-invariant per dst; |logits| < 10 so no
segment-max needed). Layer 3 commutes the output projection with
aggregation: sum(ex*h2) @ Wo == sum(ex*(h2@Wo)).
"""
import numpy as np
import ml_dtypes
from collections import deque
from contextlib import ExitStack

import concourse.bass as bass
import concourse.tile as tile
from concourse import bacc, mybir
from concourse.bass_utils import run_bass_kernel_spmd
from concourse.masks import make_identity

BF16 = ml_dtypes.bfloat16

N_NODES = 50000
IN_F = 128
N_CLASSES = 40
CORES = 8
SPR = 6250          # real dst nodes per core
SP = 6656           # slice rows per core (52 * 128)
NT = SP * CORES     # 53248 padded table rows
W = 49              # dst windows per core (ceil(6250/128))
ROW = 384           # table row elems (bf16 -> 768B, multiple of 256B)
NTILE = SP // 128   # 52 node tiles per slice
NEG_SLOPE = 0.2
EXPF = mybir.ActivationFunctionType.Exp
COPYF = mybir.ActivationFunctionType.Copy
LNF = mybir.ActivationFunctionType.Ln

HT = NTILE // 2      # 26 tiles per table half
HROWS = HT * 128     # 3328 slice rows per half
HBLK = HROWS * CORES # 26624 table rows per half
HALF = HBLK          # gather low/high split == table halves
G = 5                # high-gather lookahead (windows)

# high-half windows first, so the next table's high half completes (and
# AllGathers) mid-layer; the next layer's high gathers then run ahead while
# the low half finishes.
WORDER = list(range(26, 49)) + list(range(0, 26))
TORDER = list(range(26, 52)) + list(range(0, 26))

# h column interleave: new col k holds old col (k%8)*32 + (k//8), i.e.
# feature-major with the 8 heads contiguous. Lets per-head edge weights
# broadcast with a unit-stride inner dim of 8 on the DVE.
PERM = np.array([(k % 8) * 32 + (k // 8) for k in range(256)], np.int64)


def _remap(n):
    """Global table row for node n, laid out (half, core, tile, row) so
    half-wise partial AllGathers are contiguous in slice and table."""
    c = n // SPR
    r = n % SPR
    t = r // 128
    h = t // HT
    return h * HBLK + c * HROWS + (t % HT) * 128 + (r % 128)


def _wrap16(vals, nidx):
    """dma_gather index layout: flat idx i -> [i%16, i//16], replicated to
    all 8 groups of 16 partitions."""
    blk = np.zeros((16, nidx // 16), np.int16)
    blk[np.arange(nidx) % 16, np.arange(nidx) // 16] = vals
    return np.tile(blk, (8, 1))


def _host_prep(src, dst):
    gsrc = _remap(src.astype(np.int64))
    d64 = dst.astype(np.int64)
    core = d64 // SPR
    ld = d64 % SPR
    w = ld >> 7
    dstl = (ld & 127).astype(np.int64)
    ishigh = (gsrc >= HALF).astype(np.int64)
    wg = core * W + w

    order = np.lexsort((ishigh, wg))
    gsrc_s, dstl_s, wg_s, hi_s = gsrc[order], dstl[order], wg[order], ishigh[order]

    gkey = wg_s * 2 + hi_s
    uniq, starts, counts = np.unique(gkey, return_index=True, return_counts=True)
    pos_in_grp = np.arange(len(gkey)) - np.repeat(starts, counts)

    nlow = np.zeros(CORES * W, np.int64)
    nhigh = np.zeros(CORES * W, np.int64)
    for u, c in zip(uniq, counts):
        (nlow if u % 2 == 0 else nhigh)[u // 2] = c
    # Variable per-window gather regions (compile-time constants, shared
    # program): KL[w] = low chunks needed by the worst core for window w.
    KL = [max(1, int(np.ceil(max(nlow[c * W + w2] for c in range(CORES)) / 128)))
          for w2 in range(W)]
    KH = [max(1, int(np.ceil(max(nhigh[c * W + w2] for c in range(CORES)) / 128)))
          for w2 in range(W)]
    KW = [KL[w2] + KH[w2] for w2 in range(W)]
    # packed offsets, in chunks
    OFFL = np.concatenate([[0], np.cumsum(KL)]).astype(np.int64)
    OFFH = np.concatenate([[0], np.cumsum(KH)]).astype(np.int64)
    OFFW = np.concatenate([[0], np.cumsum(KW)]).astype(np.int64)

    w_local = wg_s % W
    # slot relative to the window's packed region
    rel = np.where(hi_s == 1, np.asarray(KL)[w_local] * 128, 0) + pos_in_grp
    slot = OFFW[w_local] * 128 + rel
    core_s = wg_s // W

    NSLOT = int(OFFW[-1]) * 128
    NLOW = int(OFFL[-1]) * 128
    NHIGH = int(OFFH[-1]) * 128
    per_core = []
    for c in range(CORES):
        m = core_s == c
        gs = gsrc_s[m]
        dl = dstl_s[m]
        hi = hi_s[m]
        wl = w_local[m]
        rl = rel[m]
        sl = slot[m]

        lowidx = np.zeros(NLOW, np.int16)
        highidx = np.zeros(NHIGH, np.int16)
        lm = hi == 0
        lowidx[OFFL[wl[lm]] * 128 + rl[lm]] = gs[lm].astype(np.int16)
        hm = hi == 1
        highidx[OFFH[wl[hm]] * 128 + (rl[hm] - np.asarray(KL)[wl[hm]] * 128)] = \
            (gs[hm] - HALF).astype(np.int16)

        gilo = np.concatenate(
            [_wrap16(lowidx[OFFL[i] * 128:OFFL[i + 1] * 128], KL[i] * 128)
             for i in range(W)], axis=1)
        gihi = np.concatenate(
            [_wrap16(highidx[OFFH[i] * 128:OFFH[i + 1] * 128], KH[i] * 128)
             for i in range(W)], axis=1)

        # one-hot streams; pad slots keep all-zero column/row -> zero contribution
        selS = np.zeros((128, NSLOT), BF16)    # sel[d, slot]
        selT = np.zeros((128, NSLOT), BF16)    # selT[e_lane, chunk*128 + d]
        e_lane = sl % 128
        chunk = sl // 128
        selS[dl, sl] = 1
        selT[e_lane, chunk * 128 + dl] = 1

        per_core.append(dict(gilo=gilo, gihi=gihi, selS=selS, selT=selT))
    return per_core, KL, KH


def _alar_block(al, ar, fout):
    """[fout, 16]: col j (<8) extracts el head j, col j+8 er head j."""
    H, F = al.shape
    m = np.zeros((fout, 16), np.float32)
    for j in range(H):
        m[j * F:(j + 1) * F, j] = al[j]
        m[j * F:(j + 1) * F, j + 8] = ar[j]
    return m


def _build_program(KL, KH):
    KW = [KL[w] + KH[w] for w in range(W)]
    OFFL = [0]
    OFFH = [0]
    OFFW = [0]
    for w in range(W):
        OFFL.append(OFFL[-1] + KL[w])
        OFFH.append(OFFH[-1] + KH[w])
        OFFW.append(OFFW[-1] + KW[w])
    CLmax, CHmax, CPWmax = max(KL), max(KH), max(KW)

    nc = bacc.Bacc("TRN2", target_bir_lowering=False, debug=False, num_devices=CORES)
    f32, bf16, i16 = mybir.dt.float32, mybir.dt.bfloat16, mybir.dt.int16

    xT_d = nc.declare_dram_parameter("xT", [IN_F, SP], bf16, isOutput=False)
    gilo_d = nc.declare_dram_parameter("gilo", [128, OFFL[-1] * 8], i16, isOutput=False)
    gihi_d = nc.declare_dram_parameter("gihi", [128, OFFH[-1] * 8], i16, isOutput=False)
    selS_d = nc.declare_dram_parameter("selS", [128, OFFW[-1] * 128], bf16, isOutput=False)
    selT_d = nc.declare_dram_parameter("selT", [128, OFFW[-1] * 128], bf16, isOutput=False)
    w1_d = nc.declare_dram_parameter("w1", [IN_F, 256], bf16, isOutput=False)
    w2_d = nc.declare_dram_parameter("w2", [128, 2, 256], bf16, isOutput=False)
    wo_d = nc.declare_dram_parameter("wo", [128, 2, N_CLASSES], bf16, isOutput=False)
    alar_d = nc.declare_dram_parameter("alar", [128, 3, 2, 16], bf16, isOutput=False)
    outy = nc.declare_dram_parameter("outy", [SP, N_CLASSES], f32, isOutput=True)

    with ExitStack() as ctx:
        tc = ctx.enter_context(tile.TileContext(nc))
        const = ctx.enter_context(tc.tile_pool(name="const", bufs=1))
        dram = ctx.enter_context(tc.tile_pool(name="dram", bufs=1, space="DRAM"))
        ghpool = ctx.enter_context(tc.tile_pool(name="ghpool", bufs=G + 1))
        glpool = ctx.enter_context(tc.tile_pool(name="glpool", bufs=3))
        spool = ctx.enter_context(tc.tile_pool(name="spool", bufs=2))
        npool = ctx.enter_context(tc.tile_pool(name="npool", bufs=2))
        pwin = ctx.enter_context(tc.tile_pool(name="pwin", bufs=2, space="PSUM"))
        ppp = ctx.enter_context(tc.tile_pool(name="ppp", bufs=2, space="PSUM"))
        pnode = ctx.enter_context(tc.tile_pool(name="pnode", bufs=3, space="PSUM"))

        Tsl_h = [dram.tile([HROWS, ROW], bf16, name=f"tsl{h}") for h in range(2)]
        # one Shared tensor per (layer, half): exactly one AllGather writer
        # each, which enables the fast no-bounce HBM-HBM collective path.
        Tlo = [dram.tile([HBLK, ROW], bf16, name=f"tlo{li}", addr_space="Shared")
               for li in range(3)]
        Thi = [dram.tile([HBLK, ROW], bf16, name=f"thi{li}", addr_space="Shared")
               for li in range(3)]

        def Tslice_rows(r0, r1):
            h = r0 // HROWS
            assert (r1 - 1) // HROWS == h
            return Tsl_h[h][r0 - h * HROWS:r1 - h * HROWS, :]

        gilo_t = const.tile([128, OFFL[-1] * 8], i16)
        nc.sync.dma_start(out=gilo_t[:], in_=gilo_d[:, :])
        gihi_t = const.tile([128, OFFH[-1] * 8], i16)
        nc.sync.dma_start(out=gihi_t[:], in_=gihi_d[:, :])
        w1_t = const.tile([IN_F, 256], bf16)
        nc.sync.dma_start(out=w1_t[:], in_=w1_d[:, :])
        w2_t = const.tile([128, 2, 256], bf16)
        nc.sync.dma_start(out=w2_t[:], in_=w2_d[:, :, :])
        wo_t = const.tile([128, 2, N_CLASSES], bf16)
        nc.sync.dma_start(out=wo_t[:], in_=wo_d[:, :, :])
        alar_t = const.tile([128, 3, 2, 16], bf16)
        nc.sync.dma_start(out=alar_t[:], in_=alar_d[:, :, :, :])
        ident = const.tile([128, 128], bf16)
        make_identity(nc, ident[:])

        # zero-fill staging buffers once: chunks beyond a window's region are
        # never written by its gather, so they must hold finite data.
        for _ in range(G + 1):
            gi = ghpool.tile([128, CHmax, ROW], bf16, tag="gh")
            nc.vector.memset(gi[:], 0.0)
        for _ in range(3):
            gi = glpool.tile([128, CLmax, ROW], bf16, tag="gl")
            nc.vector.memset(gi[:], 0.0)
        for _ in range(2):
            ee = npool.tile([128, 128], bf16, tag="eesb")
            nc.vector.memset(ee[:], 0.0)

        def emit_table_rows(al_idx, h_T, h_node_src, t):
            """Assemble table row tile [h | el | er | 0] for node rows
            [t*128,(t+1)*128) and DMA it into the slice quarter."""
            row_t = npool.tile([128, ROW], bf16, tag="row")
            nc.scalar.activation(out=row_t[:, 0:256], in_=h_node_src, func=COPYF)
            el_ps = pnode.tile([16, 128], f32, tag="nps")
            for kt in range(2):
                nc.tensor.matmul(out=el_ps[:], lhsT=alar_t[:, al_idx, kt, :],
                                 rhs=h_T[:, kt, :], start=(kt == 0), stop=(kt == 1))
            ee_sb = npool.tile([128, 128], bf16, tag="eesb")
            nc.scalar.activation(out=ee_sb[0:16, :], in_=el_ps[:], func=COPYF)
            eeT_ps = pnode.tile([128, 128], bf16, tag="nps")
            nc.tensor.transpose(out=eeT_ps[:], in_=ee_sb[:], identity=ident[:])
            nc.scalar.activation(out=row_t[:, 256:272], in_=eeT_ps[:, 0:16], func=COPYF)
            nc.sync.dma_start(out=Tslice_rows(t * 128, (t + 1) * 128), in_=row_t[:])

        def ag_half(h, li):
            dst = Thi[li] if h == 1 else Tlo[li]
            nc.gpsimd.collective_compute(
                "AllGather", mybir.AluOpType.bypass,
                replica_groups=[list(range(CORES))],
                ins=[Tsl_h[h].opt()],
                outs=[dst[:, :]])

        # ---- P0: layer-1 table from x (reversed-quarter tile order) ----
        for i, t in enumerate(TORDER):
            xT_t = npool.tile([128, 128], bf16, tag="xTt")
            nc.sync.dma_start(out=xT_t[:], in_=xT_d[:, t * 128:(t + 1) * 128])
            h_ps = pnode.tile([128, 2, 128], f32, tag="nps")
            for mt in range(2):
                nc.tensor.matmul(out=h_ps[:, mt, :], lhsT=w1_t[:, mt * 128:(mt + 1) * 128],
                                 rhs=xT_t[:], start=True, stop=True)
            h_T = npool.tile([128, 2, 128], bf16, tag="hT")
            nc.scalar.activation(out=h_T[:], in_=h_ps[:], func=COPYF)
            hb_ps = pnode.tile([128, 2, 128], bf16, tag="nps")
            for t2 in range(2):
                nc.tensor.transpose(out=hb_ps[:, t2, :], in_=h_T[:, t2, :], identity=ident[:])
            hb = npool.tile([128, 256], f32, tag="hb")
            nc.scalar.activation(out=hb[:], in_=hb_ps[:].rearrange("p a b -> p (a b)"),
                                 func=COPYF)
            emit_table_rows(0, h_T, hb[:], t)
            if i == 25:
                ag_half(1, 0)
            elif i == 51:
                ag_half(0, 0)

        # ---- 3 layers of windowed edge aggregation ----
        for l in range(3):
            gh_q = deque()

            def issue_high(w, li=l):
                gh = ghpool.tile([128, CHmax, ROW], bf16, tag="gh")
                nc.gpsimd.dma_gather(
                    out_ap=gh[:, 0:KH[w], :], in_ap=Thi[li][:, :],
                    idxs_ap=gihi_t[:, OFFH[w] * 8:OFFH[w + 1] * 8],
                    num_idxs=KH[w] * 128, num_idxs_reg=KH[w] * 128, elem_size=ROW,
                    single_packet=False)
                gh_q.append(gh)

            for j in range(G):
                issue_high(WORDER[j])

            for i, w in enumerate(WORDER):
                if i + G < W:
                    issue_high(WORDER[i + G])
                kl, kh, kw = KL[w], KH[w], KW[w]
                gl = glpool.tile([128, CLmax, ROW], bf16, tag="gl")
                nc.gpsimd.dma_gather(
                    out_ap=gl[:, 0:kl, :], in_ap=Tlo[l][:, :],
                    idxs_ap=gilo_t[:, OFFL[w] * 8:OFFL[w + 1] * 8],
                    num_idxs=kl * 128, num_idxs_reg=kl * 128, elem_size=ROW,
                    single_packet=False)
                gh = gh_q.popleft()

                selw = spool.tile([128, CPWmax, 128], bf16, tag="selw")
                nc.sync.dma_start(out=selw[:, 0:kw, :],
                                  in_=selS_d[:, OFFW[w] * 128:OFFW[w + 1] * 128])
                selTw = spool.tile([128, CPWmax, 128], bf16, tag="selTw")
                nc.sync.dma_start(out=selTw[:, 0:kw, :],
                                  in_=selT_d[:, OFFW[w] * 128:OFFW[w + 1] * 128])
                err_t = spool.tile([128, 8], bf16, tag="err")
                nc.sync.dma_start(out=err_t[:], in_=Tslice_rows(w * 128, (w + 1) * 128)[:, 264:272])

                # er of each slot's dst, expanded edge-wise via one-hot matmul
                pp_ps = ppp.tile([128, CPWmax, 8], f32, tag="pp")
                for c in range(kw):
                    nc.tensor.matmul(out=pp_ps[:, c, :], lhsT=selw[:, c, :],
                                     rhs=err_t[:], start=True, stop=True)
                # e = el_src + er_dst; ex = exp(lrelu(e)) = max(exp(e), exp(0.2e))
                ef = spool.tile([128, CPWmax, 8], f32, tag="ef")
                nc.vector.tensor_tensor(out=ef[:, 0:kl, :], in0=gl[:, 0:kl, 256:264],
                                        in1=pp_ps[:, 0:kl, :], op=mybir.AluOpType.add)
                nc.vector.tensor_tensor(out=ef[:, kl:kw, :], in0=gh[:, 0:kh, 256:264],
                                        in1=pp_ps[:, kl:kw, :], op=mybir.AluOpType.add)
                ex1 = spool.tile([128, CPWmax, 8], bf16, tag="ex1")
                nc.scalar.activation(out=ex1[:, 0:kw, :], in_=ef[:, 0:kw, :], func=EXPF)
                ex2 = spool.tile([128, CPWmax, 8], bf16, tag="ex2")
                nc.scalar.activation(out=ex2[:, 0:kw, :], in_=ef[:, 0:kw, :], func=EXPF,
                                     scale=NEG_SLOPE)
                rhs_w = spool.tile([128, CPWmax, 264], bf16, tag="rhsw")
                nc.vector.tensor_tensor(out=rhs_w[:, 0:kw, 256:264], in0=ex1[:, 0:kw, :],
                                        in1=ex2[:, 0:kw, :], op=mybir.AluOpType.max)
                if l < 2:
                    for gt, c0, c1 in ((gl, 0, kl), (gh, kl, kw)):
                        nc.vector.tensor_tensor(
                            out=rhs_w[:, c0:c1, 0:256].rearrange("p c (f h) -> p c f h", h=8),
                            in0=gt[:, 0:c1 - c0, 0:256].rearrange("p c (f h) -> p c f h", h=8),
                            in1=rhs_w[:, c0:c1, 256:264].rearrange("p c (o h) -> p c o h", o=1)
                                .to_broadcast([128, c1 - c0, 32, 8]),
                            op=mybir.AluOpType.mult)
                else:
                    for gt, c0, c1 in ((gl, 0, kl), (gh, kl, kw)):
                        nc.vector.tensor_tensor(
                            out=rhs_w[:, c0:c1, 0:256],
                            in0=gt[:, 0:c1 - c0, 0:256],
                            in1=rhs_w[:, c0:c1, 256:257].to_broadcast([128, c1 - c0, 256]),
                            op=mybir.AluOpType.mult)
                agg_ps = pwin.tile([128, 264], f32, tag="agg")
                for c in range(kw):
                    nc.tensor.matmul(out=agg_ps[:], lhsT=selTw[:, c, :], rhs=rhs_w[:, c, :],
                                     start=(c == 0), stop=(c == kw - 1))

                # ---- per-window node phase ----
                if l < 2:
                    s_rec = npool.tile([128, 8], f32, tag="srec")
                    nc.vector.tensor_scalar_add(s_rec[:], agg_ps[:, 256:264], 1e-16)
                    nc.vector.reciprocal(out=s_rec[:], in_=s_rec[:])
                    u_t = npool.tile([128, 256], f32, tag="ut")
                    nc.vector.tensor_tensor(
                        out=u_t[:].rearrange("p (f h) -> p f h", h=8),
                        in0=agg_ps[:, 0:256].rearrange("p (f h) -> p f h", h=8),
                        in1=s_rec[:].rearrange("p (o h) -> p o h", o=1).to_broadcast([128, 32, 8]),
                        op=mybir.AluOpType.mult)
                    # elu(x) = exp(min(x,0)) - 1 + relu(x)
                    m0 = npool.tile([128, 256], f32, tag="m0")
                    nc.vector.tensor_scalar_min(m0[:], u_t[:], 0.0)
                    e0 = npool.tile([128, 256], f32, tag="e0")
                    nc.scalar.activation(out=e0[:], in_=m0[:], func=EXPF)
                    r0 = npool.tile([128, 256], f32, tag="r0")
                    nc.vector.tensor_scalar_max(r0[:], u_t[:], 0.0)
                    nc.vector.tensor_tensor(out=e0[:], in0=e0[:], in1=r0[:],
                                            op=mybir.AluOpType.add)
                    nc.vector.tensor_scalar_add(e0[:], e0[:], -1.0)
                    ub = npool.tile([128, 256], bf16, tag="ub")
                    nc.scalar.activation(out=ub[:], in_=e0[:], func=COPYF)
                    uT_ps = pnode.tile([128, 2, 128], bf16, tag="nps")
                    for t2 in range(2):
                        nc.tensor.transpose(out=uT_ps[:, t2, :], in_=ub[:, t2 * 128:(t2 + 1) * 128],
                                            identity=ident[:])
                    uT = npool.tile([128, 2, 128], bf16, tag="uT")
                    nc.scalar.activation(out=uT[:], in_=uT_ps[:], func=COPYF)
                    if l == 0:
                        h_ps = pnode.tile([128, 2, 128], f32, tag="nps")
                        for mt in range(2):
                            for kt in range(2):
                                nc.tensor.matmul(out=h_ps[:, mt, :],
                                                 lhsT=w2_t[:, kt, mt * 128:(mt + 1) * 128],
                                                 rhs=uT[:, kt, :],
                                                 start=(kt == 0), stop=(kt == 1))
                        h_T = npool.tile([128, 2, 128], bf16, tag="hT")
                        nc.scalar.activation(out=h_T[:], in_=h_ps[:], func=COPYF)
                        hb_ps = pnode.tile([128, 2, 128], bf16, tag="nps")
                        for t2 in range(2):
                            nc.tensor.transpose(out=hb_ps[:, t2, :], in_=h_T[:, t2, :],
                                                identity=ident[:])
                        hb = npool.tile([128, 256], f32, tag="hb")
                        nc.scalar.activation(out=hb[:], in_=hb_ps[:].rearrange("p a b -> p (a b)"),
                                             func=COPYF)
                        emit_table_rows(1, h_T, hb[:], w)
                    else:
                        emit_table_rows(2, uT, e0[:], w)
                else:
                    s_rec = npool.tile([128, 1], f32, tag="srec3")
                    nc.vector.tensor_scalar_add(s_rec[:], agg_ps[:, 256:257], 1e-16)
                    nc.vector.reciprocal(out=s_rec[:], in_=s_rec[:])
                    u_t = npool.tile([128, 256], f32, tag="ut")
                    nc.vector.tensor_tensor(
                        out=u_t[:], in0=agg_ps[:, 0:256],
                        in1=s_rec[:].to_broadcast([128, 256]), op=mybir.AluOpType.mult)
                    ub = npool.tile([128, 256], bf16, tag="ub")
                    nc.scalar.activation(out=ub[:], in_=u_t[:], func=COPYF)
                    uT_ps = pnode.tile([128, 2, 128], bf16, tag="nps")
                    for t2 in range(2):
                        nc.tensor.transpose(out=uT_ps[:, t2, :], in_=ub[:, t2 * 128:(t2 + 1) * 128],
                                            identity=ident[:])
                    uT = npool.tile([128, 2, 128], bf16, tag="uT")
                    nc.scalar.activation(out=uT[:], in_=uT_ps[:], func=COPYF)
                    o_ps = pnode.tile([N_CLASSES, 128], f32, tag="nps")
                    for kt in range(2):
                        nc.tensor.matmul(out=o_ps[:], lhsT=wo_t[:, kt, :],
                                         rhs=uT[:, kt, :], start=(kt == 0), stop=(kt == 1))
                    ob = npool.tile([128, 128], bf16, tag="ob")
                    nc.vector.memset(ob[:], 0.0)
                    nc.vector.tensor_copy(out=ob[0:N_CLASSES, :], in_=o_ps[:])
                    on_ps = pnode.tile([128, 128], bf16, tag="nps")
                    nc.tensor.transpose(out=on_ps[:], in_=ob[:], identity=ident[:])
                    o_n = npool.tile([128, N_CLASSES], f32, tag="on")
                    nc.vector.tensor_copy(out=o_n[:], in_=on_ps[:, 0:N_CLASSES])
                    mx = npool.tile([128, 1], f32, tag="mx")
                    nc.vector.tensor_reduce(out=mx[:], in_=o_n[:], axis=mybir.AxisListType.X,
                                            op=mybir.AluOpType.max)
                    nc.vector.tensor_tensor(out=o_n[:], in0=o_n[:],
                                            in1=mx[:].to_broadcast([128, N_CLASSES]),
                                            op=mybir.AluOpType.subtract)
                    ex_t = npool.tile([128, N_CLASSES], f32, tag="ext")
                    nc.scalar.activation(out=ex_t[:], in_=o_n[:], func=EXPF)
                    sm = npool.tile([128, 1], f32, tag="sm")
                    nc.vector.tensor_reduce(out=sm[:], in_=ex_t[:], axis=mybir.AxisListType.X,
                                            op=mybir.AluOpType.add)
                    ln_t = npool.tile([128, 1], f32, tag="lnt")
                    nc.scalar.activation(out=ln_t[:], in_=sm[:], func=LNF)
                    res = npool.tile([128, N_CLASSES], f32, tag="res")
                    nc.vector.tensor_tensor(out=res[:], in0=o_n[:],
                                            in1=ln_t[:].to_broadcast([128, N_CLASSES]),
                                            op=mybir.AluOpType.subtract)
                    nc.sync.dma_start(out=outy[w * 128:(w + 1) * 128, :], in_=res[:])

                # partial AllGather overlapped with later windows
                if l < 2 and i == 24:
                    ag_half(1, l + 1)
            if l < 2:
                ag_half(0, l + 1)

    nc.compile()
    return nc


_CACHE = {}
_LAST = {}


def _make_in_maps(ins, per_core):
    x = np.asarray(ins["x"], np.float32)
    W1n = np.asarray(ins["W1"], np.float32)
    W2n = np.asarray(ins["W2"], np.float32)
    Won = np.asarray(ins["Wo"], np.float32)
    al1n, ar1n = np.asarray(ins["al1"], np.float32), np.asarray(ins["ar1"], np.float32)
    al2n, ar2n = np.asarray(ins["al2"], np.float32), np.asarray(ins["ar2"], np.float32)
    alon, aron = np.asarray(ins["alo"], np.float32), np.asarray(ins["aro"], np.float32)

    W1p = W1n[:, PERM]
    W2p = W2n[PERM][:, PERM]
    Wop = Won[PERM]

    w1_h = W1p.astype(BF16)                                          # [128, 256]
    w2_h = np.ascontiguousarray(
        W2p.reshape(2, 128, 256).transpose(1, 0, 2)).astype(BF16)    # [128, 2, 256]
    wo_h = np.ascontiguousarray(
        Wop.reshape(2, 128, N_CLASSES).transpose(1, 0, 2)).astype(BF16)
    alar = np.zeros((3, 256, 16), np.float32)
    alar[0] = _alar_block(al1n, ar1n, 256)
    alar[1] = _alar_block(al2n, ar2n, 256)
    alar[2][:, 0] = Won @ alon[0]
    alar[2][:, 8] = Won @ aron[0]
    alar = alar[:, PERM, :]
    alar_h = np.ascontiguousarray(
        alar.reshape(3, 2, 128, 16).transpose(2, 0, 1, 3)).astype(BF16)  # [128, 3, 2, 16]

    in_maps = []
    for c in range(CORES):
        xs = np.zeros((SP, IN_F), np.float32)
        xs[:SPR] = x[c * SPR:(c + 1) * SPR]
        xT_h = np.ascontiguousarray(xs.T).astype(BF16)               # [128, SP]
        pc = per_core[c]
        in_maps.append(dict(
            xT=xT_h, gilo=pc["gilo"], gihi=pc["gihi"],
            selS=pc["selS"], selT=pc["selT"],
            w1=w1_h, w2=w2_h, wo=wo_h, alar=alar_h))
    return in_maps


def kernel(x, src, dst, W1, al1, ar1, W2, al2, ar2, Wo, alo, aro):
    src = np.asarray(src, np.int32)
    dst = np.asarray(dst, np.int32)

    per_core, KL, KH = _host_prep(src, dst)

    key = (tuple(KL), tuple(KH))
    if key not in _CACHE:
        _CACHE[key] = _build_program(KL, KH)
    nc = _CACHE[key]

    in_maps = _make_in_maps(
        dict(x=x, W1=W1, al1=al1, ar1=ar1, W2=W2, al2=al2, ar2=ar2,
             Wo=Wo, alo=alo, aro=aro), per_core)

    _LAST["nc"] = nc
    _LAST["in_maps"] = in_maps
    res = run_bass_kernel_spmd(nc, in_maps, list(range(CORES)))
    out = np.concatenate([res.results[c]["outy"][:SPR] for c in range(CORES)], axis=0)
    return out.astype(np.float32)


# revision 36
# speedup vs baseline: 50855.2658x; 1.1862x over previous
"""3-layer GAT on Trainium2, 8 NeuronCores — v2.

Strategy (dst-sharded, replicated tables). The hard resource is the GpSimd
Q7 cluster: every dma_gather descriptor costs ~8-10ns of Q7 descriptor
generation, and the kernel needs one gathered table row per edge per layer
(~100k rows/core/layer). v2 therefore (a) trims pad slots via the Q7
kernel's trailing-negative-index trim, (b) keeps Q7 busy through layer
boundaries: tables AllGather in QUARTERS, windows are processed in
reversed-quarter order so early quarters of the next table finish mid-layer,
and each window's high-half gather (table rows >= 26624) runs G windows
ahead of its low-half gather, bridging the table-ready gap.

Table row (768B, bf16): [h(256, (f,h)-interleaved) | el(8) | er(8) | pad96].
Edge weights: ex = exp(leaky_relu(el_src + er_dst)) computed exactly as
max(exp(e), exp(0.2 e)). Softmax normalization happens after aggregation
(alpha = ex/sum(ex) is scale-invariant per dst; |logits| < 10 so no
segment-max needed). Layer 3 commutes the output projection with
aggregation: sum(ex*h2) @ Wo == sum(ex*(h2@Wo)).
"""
import numpy as np
import ml_dtypes
from collections import deque
from contextlib import ExitStack

import concourse.bass as bass
import concourse.tile as tile
from concourse import bacc, mybir
from concourse.bass_utils import run_bass_kernel_spmd
from concourse.masks import make_identity

BF16 = ml_dtypes.bfloat16

N_NODES = 50000
IN_F = 128
N_CLASSES = 40
CORES = 8
SPR = 6250          # real dst nodes per core
SP = 6656           # slice rows per core (52 * 128)
NT = SP * CORES     # 53248 padded table rows
W = 49              # dst windows per core (ceil(6250/128))
ROW = 384           # table row elems (bf16 -> 768B, multiple of 256B)
NTILE = SP // 128   # 52 node tiles per slice
NEG_SLOPE = 0.2
EXPF = mybir.ActivationFunctionType.Exp
COPYF = mybir.ActivationFunctionType.Copy
LNF = mybir.ActivationFunctionType.Ln

HT = NTILE // 2      # 26 tiles per table half
HROWS = HT * 128     # 3328 slice rows per half
HBLK = HROWS * CORES # 26624 table rows per half
HALF = HBLK          # gather low/high split == table halves
G = 10               # high-gather lookahead (windows)

# high-half windows first, so the next table's high half completes (and
# AllGathers) mid-layer; the next layer's high gathers then run ahead while
# the low half finishes.
WORDER = list(range(26, 49)) + list(range(0, 26))
TORDER = list(range(26, 52)) + list(range(0, 26))

# h column interleave: new col k holds old col (k%8)*32 + (k//8), i.e.
# feature-major with the 8 heads contiguous. Lets per-head edge weights
# broadcast with a unit-stride inner dim of 8 on the DVE.
PERM = np.array([(k % 8) * 32 + (k // 8) for k in range(256)], np.int64)


def _remap(n):
    """Global table row for node n, laid out (half, core, tile, row) so
    half-wise partial AllGathers are contiguous in slice and table."""
    c = n // SPR
    r = n % SPR
    t = r // 128
    h = t // HT
    return h * HBLK + c * HROWS + (t % HT) * 128 + (r % 128)


def _wrap16(vals, nidx):
    """dma_gather index layout: flat idx i -> [i%16, i//16], replicated to
    all 8 groups of 16 partitions."""
    blk = np.zeros((16, nidx // 16), np.int16)
    blk[np.arange(nidx) % 16, np.arange(nidx) // 16] = vals
    return np.tile(blk, (8, 1))


def _host_prep(src, dst):
    gsrc = _remap(src.astype(np.int64))
    d64 = dst.astype(np.int64)
    core = d64 // SPR
    ld = d64 % SPR
    w = ld >> 7
    dstl = (ld & 127).astype(np.int64)
    ishigh = (gsrc >= HALF).astype(np.int64)
    wg = core * W + w

    order = np.lexsort((ishigh, wg))
    gsrc_s, dstl_s, wg_s, hi_s = gsrc[order], dstl[order], wg[order], ishigh[order]

    gkey = wg_s * 2 + hi_s
    uniq, starts, counts = np.unique(gkey, return_index=True, return_counts=True)
    pos_in_grp = np.arange(len(gkey)) - np.repeat(starts, counts)

    nlow = np.zeros(CORES * W, np.int64)
    nhigh = np.zeros(CORES * W, np.int64)
    for u, c in zip(uniq, counts):
        (nlow if u % 2 == 0 else nhigh)[u // 2] = c
    # Variable per-window gather regions (compile-time constants, shared
    # program): KL[w] = low chunks needed by the worst core for window w.
    KL = [max(1, int(np.ceil(max(nlow[c * W + w2] for c in range(CORES)) / 128)))
          for w2 in range(W)]
    KH = [max(1, int(np.ceil(max(nhigh[c * W + w2] for c in range(CORES)) / 128)))
          for w2 in range(W)]
    KW = [KL[w2] + KH[w2] for w2 in range(W)]
    # packed offsets, in chunks
    OFFL = np.concatenate([[0], np.cumsum(KL)]).astype(np.int64)
    OFFH = np.concatenate([[0], np.cumsum(KH)]).astype(np.int64)
    OFFW = np.concatenate([[0], np.cumsum(KW)]).astype(np.int64)

    w_local = wg_s % W
    # slot relative to the window's packed region
    rel = np.where(hi_s == 1, np.asarray(KL)[w_local] * 128, 0) + pos_in_grp
    slot = OFFW[w_local] * 128 + rel
    core_s = wg_s // W

    NSLOT = int(OFFW[-1]) * 128
    NLOW = int(OFFL[-1]) * 128
    NHIGH = int(OFFH[-1]) * 128
    per_core = []
    for c in range(CORES):
        m = core_s == c
        gs = gsrc_s[m]
        dl = dstl_s[m]
        hi = hi_s[m]
        wl = w_local[m]
        rl = rel[m]
        sl = slot[m]

        lowidx = np.zeros(NLOW, np.int16)
        highidx = np.zeros(NHIGH, np.int16)
        lm = hi == 0
        lowidx[OFFL[wl[lm]] * 128 + rl[lm]] = gs[lm].astype(np.int16)
        hm = hi == 1
        highidx[OFFH[wl[hm]] * 128 + (rl[hm] - np.asarray(KL)[wl[hm]] * 128)] = \
            (gs[hm] - HALF).astype(np.int16)

        gilo = np.concatenate(
            [_wrap16(lowidx[OFFL[i] * 128:OFFL[i + 1] * 128], KL[i] * 128)
             for i in range(W)], axis=1)
        gihi = np.concatenate(
            [_wrap16(highidx[OFFH[i] * 128:OFFH[i + 1] * 128], KH[i] * 128)
             for i in range(W)], axis=1)

        # one-hot streams; pad slots keep all-zero column/row -> zero contribution
        selS = np.zeros((128, NSLOT), BF16)    # sel[d, slot]
        selT = np.zeros((128, NSLOT), BF16)    # selT[e_lane, chunk*128 + d]
        e_lane = sl % 128
        chunk = sl // 128
        selS[dl, sl] = 1
        selT[e_lane, chunk * 128 + dl] = 1

        per_core.append(dict(gilo=gilo, gihi=gihi, selS=selS, selT=selT))
    return per_core, KL, KH


def _alar_block(al, ar, fout):
    """[fout, 16]: col j (<8) extracts el head j, col j+8 er head j."""
    H, F = al.shape
    m = np.zeros((fout, 16), np.float32)
    for j in range(H):
        m[j * F:(j + 1) * F, j] = al[j]
        m[j * F:(j + 1) * F, j + 8] = ar[j]
    return m


def _build_program(KL, KH):
    KW = [KL[w] + KH[w] for w in range(W)]
    OFFL = [0]
    OFFH = [0]
    OFFW = [0]
    for w in range(W):
        OFFL.append(OFFL[-1] + KL[w])
        OFFH.append(OFFH[-1] + KH[w])
        OFFW.append(OFFW[-1] + KW[w])
    CLmax, CHmax, CPWmax = max(KL), max(KH), max(KW)

    nc = bacc.Bacc("TRN2", target_bir_lowering=False, debug=False, num_devices=CORES,
                   num_swdge_queues=4)
    f32, bf16, i16 = mybir.dt.float32, mybir.dt.bfloat16, mybir.dt.int16

    xT_d = nc.declare_dram_parameter("xT", [IN_F, SP], bf16, isOutput=False)
    gilo_d = nc.declare_dram_parameter("gilo", [128, OFFL[-1] * 8], i16, isOutput=False)
    gihi_d = nc.declare_dram_parameter("gihi", [128, OFFH[-1] * 8], i16, isOutput=False)
    selS_d = nc.declare_dram_parameter("selS", [128, OFFW[-1] * 128], bf16, isOutput=False)
    selT_d = nc.declare_dram_parameter("selT", [128, OFFW[-1] * 128], bf16, isOutput=False)
    w1_d = nc.declare_dram_parameter("w1", [IN_F, 256], bf16, isOutput=False)
    w2_d = nc.declare_dram_parameter("w2", [128, 2, 256], bf16, isOutput=False)
    wo_d = nc.declare_dram_parameter("wo", [128, 2, N_CLASSES], bf16, isOutput=False)
    alar_d = nc.declare_dram_parameter("alar", [128, 3, 2, 16], bf16, isOutput=False)
    outy = nc.declare_dram_parameter("outy", [SP, N_CLASSES], f32, isOutput=True)

    with ExitStack() as ctx:
        tc = ctx.enter_context(tile.TileContext(nc))
        const = ctx.enter_context(tc.tile_pool(name="const", bufs=1))
        dram = ctx.enter_context(tc.tile_pool(name="dram", bufs=1, space="DRAM"))
        ghpool = ctx.enter_context(tc.tile_pool(name="ghpool", bufs=G + 1))
        glpool = ctx.enter_context(tc.tile_pool(name="glpool", bufs=3))
        spool = ctx.enter_context(tc.tile_pool(name="spool", bufs=2))
        npool = ctx.enter_context(tc.tile_pool(name="npool", bufs=2))
        pwin = ctx.enter_context(tc.tile_pool(name="pwin", bufs=2, space="PSUM"))
        ppp = ctx.enter_context(tc.tile_pool(name="ppp", bufs=2, space="PSUM"))
        pnode = ctx.enter_context(tc.tile_pool(name="pnode", bufs=3, space="PSUM"))

        Tsl_h = [dram.tile([HROWS, ROW], bf16, name=f"tsl{h}") for h in range(2)]
        # one Shared tensor per (layer, half): exactly one AllGather writer
        # each, which enables the fast no-bounce HBM-HBM collective path.
        Tlo = [dram.tile([HBLK, ROW], bf16, name=f"tlo{li}", addr_space="Shared")
               for li in range(3)]
        Thi = [dram.tile([HBLK, ROW], bf16, name=f"thi{li}", addr_space="Shared")
               for li in range(3)]

        def Tslice_rows(r0, r1):
            h = r0 // HROWS
            assert (r1 - 1) // HROWS == h
            return Tsl_h[h][r0 - h * HROWS:r1 - h * HROWS, :]

        gilo_t = const.tile([128, OFFL[-1] * 8], i16)
        nc.sync.dma_start(out=gilo_t[:], in_=gilo_d[:, :])
        gihi_t = const.tile([128, OFFH[-1] * 8], i16)
        nc.sync.dma_start(out=gihi_t[:], in_=gihi_d[:, :])
        w1_t = const.tile([IN_F, 256], bf16)
        nc.sync.dma_start(out=w1_t[:], in_=w1_d[:, :])
        w2_t = const.tile([128, 2, 256], bf16)
        nc.sync.dma_start(out=w2_t[:], in_=w2_d[:, :, :])
        wo_t = const.tile([128, 2, N_CLASSES], bf16)
        nc.sync.dma_start(out=wo_t[:], in_=wo_d[:, :, :])
        alar_t = const.tile([128, 3, 2, 16], bf16)
        nc.sync.dma_start(out=alar_t[:], in_=alar_d[:, :, :, :])
        ident = const.tile([128, 128], bf16)
        make_identity(nc, ident[:])

        # zero-fill staging buffers once: chunks beyond a window's region are
        # never written by its gather, so they must hold finite data.
        for _ in range(G + 1):
            gi = ghpool.tile([128, CHmax, ROW], bf16, tag="gh")
            nc.vector.memset(gi[:], 0.0)
        for _ in range(3):
            gi = glpool.tile([128, CLmax, ROW], bf16, tag="gl")
            nc.vector.memset(gi[:], 0.0)
        for _ in range(2):
            ee = npool.tile([128, 128], bf16, tag="eesb")
            nc.vector.memset(ee[:], 0.0)

        def emit_table_rows(al_idx, h_T, h_node_src, t, hbias=0.0):
            """Assemble table row tile [h | el | er | 0] for node rows
            [t*128,(t+1)*128) and DMA it into the slice quarter."""
            row_t = npool.tile([128, ROW], bf16, tag="row")
            nc.scalar.activation(out=row_t[:, 0:256], in_=h_node_src, func=COPYF,
                                 bias=hbias)
            el_ps = pnode.tile([16, 128], f32, tag="nps")
            for kt in range(2):
                nc.tensor.matmul(out=el_ps[:], lhsT=alar_t[:, al_idx, kt, :],
                                 rhs=h_T[:, kt, :], start=(kt == 0), stop=(kt == 1))
            ee_sb = npool.tile([128, 128], bf16, tag="eesb")
            nc.scalar.activation(out=ee_sb[0:16, :], in_=el_ps[:], func=COPYF)
            eeT_ps = pnode.tile([128, 128], bf16, tag="nps")
            nc.tensor.transpose(out=eeT_ps[:], in_=ee_sb[:], identity=ident[:])
            nc.scalar.activation(out=row_t[:, 256:272], in_=eeT_ps[:, 0:16], func=COPYF)
            nc.sync.dma_start(out=Tslice_rows(t * 128, (t + 1) * 128), in_=row_t[:])

        def ag_half(h, li):
            dst = Thi[li] if h == 1 else Tlo[li]
            nc.gpsimd.collective_compute(
                "AllGather", mybir.AluOpType.bypass,
                replica_groups=[list(range(CORES))],
                ins=[Tsl_h[h].opt()],
                outs=[dst[:, :]])

        # ---- P0: layer-1 table from x (reversed-quarter tile order) ----
        for i, t in enumerate(TORDER):
            xT_t = npool.tile([128, 128], bf16, tag="xTt")
            nc.sync.dma_start(out=xT_t[:], in_=xT_d[:, t * 128:(t + 1) * 128])
            h_ps = pnode.tile([128, 2, 128], f32, tag="nps")
            for mt in range(2):
                nc.tensor.matmul(out=h_ps[:, mt, :], lhsT=w1_t[:, mt * 128:(mt + 1) * 128],
                                 rhs=xT_t[:], start=True, stop=True)
            h_T = npool.tile([128, 2, 128], bf16, tag="hT")
            nc.scalar.activation(out=h_T[:], in_=h_ps[:], func=COPYF)
            hb_ps = pnode.tile([128, 2, 128], bf16, tag="nps")
            for t2 in range(2):
                nc.tensor.transpose(out=hb_ps[:, t2, :], in_=h_T[:, t2, :], identity=ident[:])
            hb = npool.tile([128, 256], f32, tag="hb")
            nc.scalar.activation(out=hb[:], in_=hb_ps[:].rearrange("p a b -> p (a b)"),
                                 func=COPYF)
            emit_table_rows(0, h_T, hb[:], t)
            if i == 25:
                ag_half(1, 0)
            elif i == 51:
                ag_half(0, 0)

        # ---- 3 layers of windowed edge aggregation ----
        for l in range(3):
            gh_q = deque()

            def issue_high(w, li=l):
                gh = ghpool.tile([128, CHmax, ROW], bf16, tag="gh")
                nc.gpsimd.dma_gather(
                    out_ap=gh[:, 0:KH[w], :], in_ap=Thi[li][:, :],
                    idxs_ap=gihi_t[:, OFFH[w] * 8:OFFH[w + 1] * 8],
                    num_idxs=KH[w] * 128, num_idxs_reg=KH[w] * 128, elem_size=ROW,
                    single_packet=False, queue_num=2 + w % 2)
                gh_q.append(gh)

            for j in range(G):
                issue_high(WORDER[j])

            for i, w in enumerate(WORDER):
                if i + G < W:
                    issue_high(WORDER[i + G])
                kl, kh, kw = KL[w], KH[w], KW[w]
                gl = glpool.tile([128, CLmax, ROW], bf16, tag="gl")
                nc.gpsimd.dma_gather(
                    out_ap=gl[:, 0:kl, :], in_ap=Tlo[l][:, :],
                    idxs_ap=gilo_t[:, OFFL[w] * 8:OFFL[w + 1] * 8],
                    num_idxs=kl * 128, num_idxs_reg=kl * 128, elem_size=ROW,
                    single_packet=False, queue_num=w % 2)
                gh = gh_q.popleft()

                selw = spool.tile([128, CPWmax, 128], bf16, tag="selw")
                nc.sync.dma_start(out=selw[:, 0:kw, :],
                                  in_=selS_d[:, OFFW[w] * 128:OFFW[w + 1] * 128])
                selTw = spool.tile([128, CPWmax, 128], bf16, tag="selTw")
                nc.sync.dma_start(out=selTw[:, 0:kw, :],
                                  in_=selT_d[:, OFFW[w] * 128:OFFW[w + 1] * 128])
                err_t = spool.tile([128, 8], bf16, tag="err")
                nc.sync.dma_start(out=err_t[:], in_=Tslice_rows(w * 128, (w + 1) * 128)[:, 264:272])

                # er of each slot's dst, expanded edge-wise via one-hot matmul
                pp_ps = ppp.tile([128, CPWmax, 8], f32, tag="pp")
                for c in range(kw):
                    nc.tensor.matmul(out=pp_ps[:, c, :], lhsT=selw[:, c, :],
                                     rhs=err_t[:], start=True, stop=True)
                # e = el_src + er_dst; ex = exp(lrelu(e)) = max(exp(e), exp(0.2e))
                ef = spool.tile([128, CPWmax, 8], f32, tag="ef")
                nc.vector.tensor_tensor(out=ef[:, 0:kl, :], in0=gl[:, 0:kl, 256:264],
                                        in1=pp_ps[:, 0:kl, :], op=mybir.AluOpType.add)
                nc.vector.tensor_tensor(out=ef[:, kl:kw, :], in0=gh[:, 0:kh, 256:264],
                                        in1=pp_ps[:, kl:kw, :], op=mybir.AluOpType.add)
                ex1 = spool.tile([128, CPWmax, 8], bf16, tag="ex1")
                nc.scalar.activation(out=ex1[:, 0:kw, :], in_=ef[:, 0:kw, :], func=EXPF)
                ex2 = spool.tile([128, CPWmax, 8], bf16, tag="ex2")
                nc.scalar.activation(out=ex2[:, 0:kw, :], in_=ef[:, 0:kw, :], func=EXPF,
                                     scale=NEG_SLOPE)
                rhs_w = spool.tile([128, CPWmax, 264], bf16, tag="rhsw")
                nc.vector.tensor_tensor(out=rhs_w[:, 0:kw, 256:264], in0=ex1[:, 0:kw, :],
                                        in1=ex2[:, 0:kw, :], op=mybir.AluOpType.max)
                if l < 2:
                    for gt, c0, c1 in ((gl, 0, kl), (gh, kl, kw)):
                        nc.vector.tensor_tensor(
                            out=rhs_w[:, c0:c1, 0:256].rearrange("p c (f h) -> p c f h", h=8),
                            in0=gt[:, 0:c1 - c0, 0:256].rearrange("p c (f h) -> p c f h", h=8),
                            in1=rhs_w[:, c0:c1, 256:264].rearrange("p c (o h) -> p c o h", o=1)
                                .to_broadcast([128, c1 - c0, 32, 8]),
                            op=mybir.AluOpType.mult)
                else:
                    for gt, c0, c1 in ((gl, 0, kl), (gh, kl, kw)):
                        nc.vector.tensor_tensor(
                            out=rhs_w[:, c0:c1, 0:256],
                            in0=gt[:, 0:c1 - c0, 0:256],
                            in1=rhs_w[:, c0:c1, 256:257].to_broadcast([128, c1 - c0, 256]),
                            op=mybir.AluOpType.mult)
                agg_ps = pwin.tile([128, 264], f32, tag="agg")
                for c in range(kw):
                    nc.tensor.matmul(out=agg_ps[:], lhsT=selTw[:, c, :], rhs=rhs_w[:, c, :],
                                     start=(c == 0), stop=(c == kw - 1))

                # ---- per-window node phase ----
                if l < 2:
                    s_rec = npool.tile([128, 8], f32, tag="srec")
                    nc.vector.tensor_scalar_add(s_rec[:], agg_ps[:, 256:264], 1e-16)
                    nc.vector.reciprocal(out=s_rec[:], in_=s_rec[:])
                    u_t = npool.tile([128, 256], f32, tag="ut")
                    nc.vector.tensor_tensor(
                        out=u_t[:].rearrange("p (f h) -> p f h", h=8),
                        in0=agg_ps[:, 0:256].rearrange("p (f h) -> p f h", h=8),
                        in1=s_rec[:].rearrange("p (o h) -> p o h", o=1).to_broadcast([128, 32, 8]),
                        op=mybir.AluOpType.mult)
                    # elu(x) + 1 = min(exp(x), 1) + relu(x); the -1 rides the
                    # downstream Copy activations as a bias.
                    e0 = npool.tile([128, 256], f32, tag="e0")
                    nc.scalar.activation(out=e0[:], in_=u_t[:], func=EXPF)
                    r0 = npool.tile([128, 256], f32, tag="r0")
                    nc.vector.tensor_scalar_max(r0[:], u_t[:], 0.0)
                    nc.vector.tensor_scalar_min(e0[:], e0[:], 1.0)
                    nc.vector.tensor_tensor(out=e0[:], in0=e0[:], in1=r0[:],
                                            op=mybir.AluOpType.add)
                    ub = npool.tile([128, 256], bf16, tag="ub")
                    nc.scalar.activation(out=ub[:], in_=e0[:], func=COPYF, bias=-1.0)
                    uT_ps = pnode.tile([128, 2, 128], bf16, tag="nps")
                    for t2 in range(2):
                        nc.tensor.transpose(out=uT_ps[:, t2, :], in_=ub[:, t2 * 128:(t2 + 1) * 128],
                                            identity=ident[:])
                    uT = npool.tile([128, 2, 128], bf16, tag="uT")
                    nc.scalar.activation(out=uT[:], in_=uT_ps[:], func=COPYF)
                    if l == 0:
                        h_ps = pnode.tile([128, 2, 128], f32, tag="nps")
                        for mt in range(2):
                            for kt in range(2):
                                nc.tensor.matmul(out=h_ps[:, mt, :],
                                                 lhsT=w2_t[:, kt, mt * 128:(mt + 1) * 128],
                                                 rhs=uT[:, kt, :],
                                                 start=(kt == 0), stop=(kt == 1))
                        h_T = npool.tile([128, 2, 128], bf16, tag="hT")
                        nc.scalar.activation(out=h_T[:], in_=h_ps[:], func=COPYF)
                        hb_ps = pnode.tile([128, 2, 128], bf16, tag="nps")
                        for t2 in range(2):
                            nc.tensor.transpose(out=hb_ps[:, t2, :], in_=h_T[:, t2, :],
                                                identity=ident[:])
                        hb = npool.tile([128, 256], f32, tag="hb")
                        nc.scalar.activation(out=hb[:], in_=hb_ps[:].rearrange("p a b -> p (a b)"),
                                             func=COPYF)
                        emit_table_rows(1, h_T, hb[:], w)
                    else:
                        emit_table_rows(2, uT, e0[:], w, hbias=-1.0)
                else:
                    s_rec = npool.tile([128, 1], f32, tag="srec3")
                    nc.vector.tensor_scalar_add(s_rec[:], agg_ps[:, 256:257], 1e-16)
                    nc.vector.reciprocal(out=s_rec[:], in_=s_rec[:])
                    u_t = npool.tile([128, 256], f32, tag="ut")
                    nc.vector.tensor_tensor(
                        out=u_t[:], in0=agg_ps[:, 0:256],
                        in1=s_rec[:].to_broadcast([128, 256]), op=mybir.AluOpType.mult)
                    ub = npool.tile([128, 256], bf16, tag="ub")
                    nc.scalar.activation(out=ub[:], in_=u_t[:], func=COPYF)
                    uT_ps = pnode.tile([128, 2, 128], bf16, tag="nps")
                    for t2 in range(2):
                        nc.tensor.transpose(out=uT_ps[:, t2, :], in_=ub[:, t2 * 128:(t2 + 1) * 128],
                                            identity=ident[:])
                    uT = npool.tile([128, 2, 128], bf16, tag="uT")
                    nc.scalar.activation(out=uT[:], in_=uT_ps[:], func=COPYF)
                    o_ps = pnode.tile([N_CLASSES, 128], f32, tag="nps")
                    for kt in range(2):
                        nc.tensor.matmul(out=o_ps[:], lhsT=wo_t[:, kt, :],
                                         rhs=uT[:, kt, :], start=(kt == 0), stop=(kt == 1))
                    ob = npool.tile([128, 128], bf16, tag="ob")
                    nc.vector.memset(ob[:], 0.0)
                    nc.vector.tensor_copy(out=ob[0:N_CLASSES, :], in_=o_ps[:])
                    on_ps = pnode.tile([128, 128], bf16, tag="nps")
                    nc.tensor.transpose(out=on_ps[:], in_=ob[:], identity=ident[:])
                    o_n = npool.tile([128, N_CLASSES], f32, tag="on")
                    nc.vector.tensor_copy(out=o_n[:], in_=on_ps[:, 0:N_CLASSES])
                    mx = npool.tile([128, 1], f32, tag="mx")
                    nc.vector.tensor_reduce(out=mx[:], in_=o_n[:], axis=mybir.AxisListType.X,
                                            op=mybir.AluOpType.max)
                    nc.vector.tensor_tensor(out=o_n[:], in0=o_n[:],
                                            in1=mx[:].to_broadcast([128, N_CLASSES]),
                                            op=mybir.AluOpType.subtract)
                    ex_t = npool.tile([128, N_CLASSES], f32, tag="ext")
                    nc.scalar.activation(out=ex_t[:], in_=o_n[:], func=EXPF)
                    sm = npool.tile([128, 1], f32, tag="sm")
                    nc.vector.tensor_reduce(out=sm[:], in_=ex_t[:], axis=mybir.AxisListType.X,
                                            op=mybir.AluOpType.add)
                    ln_t = npool.tile([128, 1], f32, tag="lnt")
                    nc.scalar.activation(out=ln_t[:], in_=sm[:], func=LNF)
                    res = npool.tile([128, N_CLASSES], f32, tag="res")
                    nc.vector.tensor_tensor(out=res[:], in0=o_n[:],
                                            in1=ln_t[:].to_broadcast([128, N_CLASSES]),
                                            op=mybir.AluOpType.subtract)
                    nc.sync.dma_start(out=outy[w * 128:(w + 1) * 128, :], in_=res[:])

                # partial AllGather overlapped with later windows
                if l < 2 and i == 24:
                    ag_half(1, l + 1)
            if l < 2:
                ag_half(0, l + 1)

    nc.compile()
    return nc


_CACHE = {}
_LAST = {}


def _make_in_maps(ins, per_core):
    x = np.asarray(ins["x"], np.float32)
    W1n = np.asarray(ins["W1"], np.float32)
    W2n = np.asarray(ins["W2"], np.float32)
    Won = np.asarray(ins["Wo"], np.float32)
    al1n, ar1n = np.asarray(ins["al1"], np.float32), np.asarray(ins["ar1"], np.float32)
    al2n, ar2n = np.asarray(ins["al2"], np.float32), np.asarray(ins["ar2"], np.float32)
    alon, aron = np.asarray(ins["alo"], np.float32), np.asarray(ins["aro"], np.float32)

    W1p = W1n[:, PERM]
    W2p = W2n[PERM][:, PERM]
    Wop = Won[PERM]

    w1_h = W1p.astype(BF16)                                          # [128, 256]
    w2_h = np.ascontiguousarray(
        W2p.reshape(2, 128, 256).transpose(1, 0, 2)).astype(BF16)    # [128, 2, 256]
    wo_h = np.ascontiguousarray(
        Wop.reshape(2, 128, N_CLASSES).transpose(1, 0, 2)).astype(BF16)
    alar = np.zeros((3, 256, 16), np.float32)
    alar[0] = _alar_block(al1n, ar1n, 256)
    alar[1] = _alar_block(al2n, ar2n, 256)
    alar[2][:, 0] = Won @ alon[0]
    alar[2][:, 8] = Won @ aron[0]
    alar = alar[:, PERM, :]
    alar_h = np.ascontiguousarray(
        alar.reshape(3, 2, 128, 16).transpose(2, 0, 1, 3)).astype(BF16)  # [128, 3, 2, 16]

    in_maps = []
    for c in range(CORES):
        xs = np.zeros((SP, IN_F), np.float32)
        xs[:SPR] = x[c * SPR:(c + 1) * SPR]
        xT_h = np.ascontiguousarray(xs.T).astype(BF16)               # [128, SP]
        pc = per_core[c]
        in_maps.append(dict(
            xT=xT_h, gilo=pc["gilo"], gihi=pc["gihi"],
            selS=pc["selS"], selT=pc["selT"],
            w1=w1_h, w2=w2_h, wo=wo_h, alar=alar_h))
    return in_maps


def kernel(x, src, dst, W1, al1, ar1, W2, al2, ar2, Wo, alo, aro):
    src = np.asarray(src, np.int32)
    dst = np.asarray(dst, np.int32)

    per_core, KL, KH = _host_prep(src, dst)

    key = (tuple(KL), tuple(KH))
    if key not in _CACHE:
        _CACHE[key] = _build_program(KL, KH)
    nc = _CACHE[key]

    in_maps = _make_in_maps(
        dict(x=x, W1=W1, al1=al1, ar1=ar1, W2=W2, al2=al2, ar2=ar2,
             Wo=Wo, alo=alo, aro=aro), per_core)

    _LAST["nc"] = nc
    _LAST["in_maps"] = in_maps
    res = run_bass_kernel_spmd(nc, in_maps, list(range(CORES)))
    out = np.concatenate([res.results[c]["outy"][:SPR] for c in range(CORES)], axis=0)
    return out.astype(np.float32)
